# revision 1
# baseline (speedup 1.0000x reference)
"""T5-style MultiHeadAttention (relative position bias) on 8 Trainium2 cores.

Sharding: core c = (b, g) with b = c // 4 (batch), g = c % 4 (head group of 4
heads).  Each core computes q/k/v projections for its 4 heads, attention with
the relative-position bias, and a partial output projection (rows of Wo for
its heads).  Host sums the 4 partials per batch element.

Key layout choices (per core):
  - x is passed transposed: xT [1024, 2048] so projections contract over
    partitions directly.
  - Q_t, K_t stored as [d, seq] (d on partitions); scores computed
    *transposed* as S_t[k, q] = K_t^T-stationary matmul, so that exp(S_t) can
    be used directly as the stationary operand of the attn@V matmul (which
    contracts over k = partitions).
  - Softmax denominator Z[q] = sum_k exp(S_t[k, q]) falls out of the attn@V
    matmul for free via a ones-column appended to V (M=65 per head).
  - No max-subtraction: scores are O(50) at most, exp stays finite in fp32
    and bf16.
  - Relative-position bias applied multiplicatively after exp:
    exp(s + bias) = exp(s) * exp(bias).  bias[k, q] = v_h[k - q + 2047] is
    Toeplitz, so exp(bias) tiles are overlapping windows of a single
    [128, 3968] sliding table per head, precomputed on host:
      T_h[p, i] = exp(v_h[p + 3967 - i])
    and the tile for (k0 = kc*128, q0 = qb*512) is T_h[:, base:base+512] with
    base = 1920 - k0 + q0.
  - Matmuls run in float32r (full PE rate at N>=256); the attention
    probabilities / V use bf16 (configurable) for the 2x DVE multiply mode.
"""

import numpy as np
from contextlib import ExitStack

import concourse.bass as bass
import concourse.tile as tile
from concourse import bacc, mybir
from concourse.bass_utils import run_bass_kernel_spmd

# ---------------------------------------------------------------- constants
B, S, D_MODEL, N_HEADS, D_KV = 2, 2048, 1024, 16, 64
NUM_BUCKETS, MAX_DIST = 32, 128
N_CORES = 8
HPC = N_HEADS // (N_CORES // B)  # heads per core = 4
DH = HPC * D_KV                  # 256 d-cols per core
TBL = 3968                       # exp-bias sliding table width
QB = 512                         # q block (free dim of score tiles)
KC = 128                         # k chunk (partition dim of score tiles)

F32 = mybir.dt.float32
F32R = mybir.dt.float32r
BF16 = mybir.dt.bfloat16
AF = mybir.ActivationFunctionType

# attention-probability dtype: BF16 (fast DVE 2x) or F32 (accurate, 1x DVE)
ATT_DT = BF16

_cache = {}


# ------------------------------------------------------------- host helpers
def _rel_bucket(d):
    """Bucket of relative position d = k - q (bidirectional T5), numpy fp32
    mirror of the jax reference."""
    nb = NUM_BUCKETS // 2
    n = -d
    ret = (n < 0).astype(np.int32) * nb
    n = np.abs(n)
    max_exact = nb // 2
    is_small = n < max_exact
    nf = np.maximum(n, 1).astype(np.float32)
    val = (
        np.log(nf / np.float32(max_exact))
        / np.float32(np.log(MAX_DIST / max_exact))
        * np.float32(nb - max_exact)
    ).astype(np.int32) + max_exact
    val = np.minimum(val, nb - 1)
    return ret + np.where(is_small, n, val)


def _expbias_tables(rel_emb):
    """[N_HEADS, 128, TBL] exp-bias sliding tables (float32)."""
    d = np.arange(-(S - 1), S)  # k - q in [-2047, 2047]
    buck = _rel_bucket(d)  # [4095]
    vals = rel_emb[buck, :].astype(np.float32)  # [4095, H]
    idx = np.arange(KC)[:, None] + (TBL - 1) - np.arange(TBL)[None, :]
    t = np.exp(vals[idx, :])  # [128, TBL, H]
    return np.ascontiguousarray(np.transpose(t, (2, 0, 1)))


# ------------------------------------------------------------- kernel body
def mha_body(tc, outs, ins, ckpt=None):
    nc = tc.nc
    ctx = ExitStack()
    xt_d = ins["xt"].bitcast(F32R)        # [1024, 2048]
    wq_d = ins["wq"].bitcast(F32R)        # [1024, 256]
    wk_d = ins["wk"].bitcast(F32R)        # [1024, 256]
    wv_d = ins["wv"].bitcast(F32R)        # [1024, 256]
    wo_d = ins["wo"].bitcast(F32R)        # [256, 1024]
    eb_d = ins["expb"]      # [HPC, 128, TBL] ATT_DT
    out_d = outs["out"]     # [2048, 1024] f32

    att_np = ATT_DT
    DKN = D_MODEL // 128    # 8 contraction chunks
    NQ = S // QB            # 4 q blocks
    NK = S // KC            # 16 k chunks

    def r(ap):  # operands already float32r
        return ap

    with ctx:
        const = ctx.enter_context(tc.tile_pool(name="const", bufs=1))

        # ---- persistent SBUF tensors
        qt = [const.tile([128, S], F32R, tag=f"qt{i}", name=f"qt{i}") for i in range(2)]
        kt = [const.tile([128, S], F32R, tag=f"kt{i}", name=f"kt{i}") for i in range(2)]
        # V with a ones column per head: [k, 4*65]; bf16 (AV stationary)
        vsb = [const.tile([128, HPC * 65], att_np, tag=f"v{i}", name=f"v{i}") for i in range(NK)]
        # normalized attention outputs, head-pairs stacked on partitions
        ust = [const.tile([128, S], F32R, tag=f"ust{i}", name=f"ust{i}") for i in range(2)]
        wo = [const.tile([128, D_MODEL], F32R, tag=f"wo{i}", name=f"wo{i}") for i in range(2)]
        ebp = ctx.enter_context(tc.tile_pool(name="expb", bufs=2))

        for i in range(2):
            nc.sync.dma_start(out=wo[i], in_=wo_d[i * 128:(i + 1) * 128, :])
        # prefetch head-pair 0's exp-bias tables before phase 1 so the
        # attention pipeline never stalls the PE at the phase boundary
        ebs = {}
        for j in range(2):
            eb = ebp.tile([128, TBL], att_np, tag=f"eb{j}", name=f"eb0_{j}")
            nc.sync.dma_start(out=eb, in_=eb_d[j])
            ebs[(0, j)] = eb

        # ---- flat pools (no scoped release: pool-release barriers idle the
        # PE >3.4us at phase boundaries and drop the HAM clock to 1.2GHz)
        wpool = ctx.enter_context(tc.tile_pool(name="wqkv", bufs=1))
        xtp = ctx.enter_context(tc.tile_pool(name="xts", bufs=6))
        xtvp = ctx.enter_context(tc.tile_pool(name="xtv", bufs=6))
        esp = ctx.enter_context(tc.tile_pool(name="es", bufs=4))
        esbp = ctx.enter_context(tc.tile_pool(name="esb", bufs=4))
        rzp = ctx.enter_context(tc.tile_pool(name="rz", bufs=2))
        outp = ctx.enter_context(tc.tile_pool(name="outsb", bufs=3))
        # one PSUM pool, 4 tags x 2 bufs of [128,512]f32 = 8 banks, shared by
        # all phases (no psum pool release barriers)
        pp = ctx.enter_context(tc.tile_pool(name="pp", bufs=2, space="PSUM"))

        # ================= phase 1: projections =================
        wq = [wpool.tile([128, DH], F32R, tag=f"wq{i}", name=f"wq{i}") for i in range(DKN)]
        wk = [wpool.tile([128, DH], F32R, tag=f"wk{i}", name=f"wk{i}") for i in range(DKN)]
        wv = [wpool.tile([128, DH], F32R, tag=f"wv{i}", name=f"wv{i}") for i in range(DKN)]
        for i in range(DKN):
            nc.sync.dma_start(out=wq[i], in_=wq_d[i * 128:(i + 1) * 128, :])
            nc.sync.dma_start(out=wk[i], in_=wk_d[i * 128:(i + 1) * 128, :])
            nc.sync.dma_start(out=wv[i], in_=wv_d[i * 128:(i + 1) * 128, :])

        # QK pass: xT streamed once; all 4 projections accumulate per block
        for qb in range(NQ):
            pq = [pp.tile([128, QB], F32, tag=t, name=f"pq{m}_{qb}")
                  for m, t in ((0, "a"), (1, "b"))]
            pk = [pp.tile([128, QB], F32, tag=t, name=f"pk{m}_{qb}")
                  for m, t in ((0, "c"), (1, "d"))]
            for dk in range(DKN):
                xtt = xtp.tile([128, QB], F32R, tag="xts", name=f"xt_{qb}_{dk}")
                nc.sync.dma_start(
                    out=xtt,
                    in_=xt_d[dk * 128:(dk + 1) * 128, qb * QB:(qb + 1) * QB])
                for m in range(2):
                    nc.tensor.matmul(
                        pq[m], wq[dk][:, m * 128:(m + 1) * 128], xtt,
                        start=(dk == 0), stop=(dk == DKN - 1))
                    nc.tensor.matmul(
                        pk[m], wk[dk][:, m * 128:(m + 1) * 128], xtt,
                        start=(dk == 0), stop=(dk == DKN - 1))
            for m in range(2):
                nc.scalar.copy(out=qt[m][:, qb * QB:(qb + 1) * QB], in_=pq[m])
                nc.scalar.copy(out=kt[m][:, qb * QB:(qb + 1) * QB], in_=pk[m])

        # V pass: xT re-streamed as [128,128] stationary tiles
        for kc in range(NK):
            pv = pp.tile([128, DH], F32, tag="abcd"[kc % 4], name=f"pv{kc}")
            for dk in range(DKN):
                xtt = xtvp.tile([128, 128], F32R, tag="xtv",
                                name=f"xtv_{kc}_{dk}")
                nc.sync.dma_start(
                    out=xtt,
                    in_=xt_d[dk * 128:(dk + 1) * 128, kc * 128:(kc + 1) * 128])
                nc.tensor.matmul(pv, xtt, wv[dk],
                                 start=(dk == 0), stop=(dk == DKN - 1))
            v3 = vsb[kc].rearrange("p (h c) -> p h c", h=HPC)
            nc.scalar.copy(
                out=v3[:, :, 0:64],
                in_=pv.rearrange("p (h c) -> p h c", h=HPC))
            nc.vector.memset(v3[:, :, 64:65], 1.0)

        # ================= phase 2+3: attention =================
        # Heads in pairs (2hp, 2hp+1): score matmuls use disjoint PE row
        # groups (rows 0-63 / 64-127) so weight loads + streaming overlap.
        mulc = 0  # round-robin DVE/GpSimd mult offload
        for hp in range(2):
            if hp == 0:
                # prefetch pair 1's tables during pair 0's compute
                for j in range(2):
                    eb = ebp.tile([128, TBL], att_np, tag=f"eb{j}",
                                  name=f"eb1_{j}")
                    nc.sync.dma_start(out=eb, in_=eb_d[2 + j])
                    ebs[(1, j)] = eb
            for qb in range(NQ):
                pus = [pp.tile([65, QB], F32, tag=t, name=f"pu{j}_{hp}_{qb}")
                       for j, t in ((0, "c"), (1, "d"))]
                for kc in range(NK):
                    base = (TBL - S) - kc * 128 + qb * QB
                    pss, ess, esbs = [], [], []
                    for j in range(2):
                        prow = slice(j * 64, j * 64 + 64)
                        ps = pp.tile([128, QB], F32, tag="ab"[j],
                                     name=f"ps{j}_{kc}")
                        nc.tensor.matmul(
                            ps,
                            kt[hp][prow, kc * 128:(kc + 1) * 128],
                            qt[hp][prow, qb * QB:(qb + 1) * QB],
                            start=True, stop=True)
                        pss.append(ps)
                    for j in range(2):
                        es = esp.tile([128, QB], att_np, tag=f"es{j}",
                                      name=f"es{j}_{kc}")
                        nc.scalar.activation(out=es, in_=pss[j], func=AF.Exp)
                        ess.append(es)
                    for j in range(2):
                        esb = esbp.tile([128, QB], att_np, tag=f"esb{j}",
                                        name=f"esb{j}_{kc}")
                        eng = nc.gpsimd if (mulc % 3 == 2) else nc.vector
                        eng.tensor_mul(esb, ess[j],
                                       ebs[(hp, j)][:, base:base + QB])
                        mulc += 1
                        esbs.append(esb)
                    for j in range(2):
                        h = hp * 2 + j
                        nc.tensor.matmul(
                            pus[j], vsb[kc][:, h * 65:(h + 1) * 65], esbs[j],
                            start=(kc == 0), stop=(kc == NK - 1))
                # normalize U[d, q] / Z[q]; Z = row 64 of pu
                for j in range(2):
                    rz = rzp.tile([1, QB], F32, tag=f"rz{j}", name=f"rz{j}_{qb}")
                    nc.vector.reciprocal(out=rz, in_=pus[j][64:65, :])
                    rzb = rzp.tile([64, QB], F32, tag=f"rzb{j}",
                                   name=f"rzb{j}_{qb}")
                    nc.gpsimd.partition_broadcast(rzb, rz, channels=64)
                    if j == 0:
                        nc.vector.tensor_mul(
                            ust[hp][0:64, qb * QB:(qb + 1) * QB],
                            pus[j][0:64, :], rzb)
                    else:
                        # DVE lanes are partition-locked; write via a [64,512]
                        # staging tile then DMA to rows 64-127
                        stg = rzp.tile([64, QB], F32R, tag="stg",
                                       name=f"stg{hp}_{qb}")
                        nc.vector.tensor_mul(stg, pus[j][0:64, :], rzb)
                        nc.sync.dma_start(
                            out=ust[hp][64:128, qb * QB:(qb + 1) * QB],
                            in_=stg)

        # ================= phase 4: output projection =================
        for qc in range(S // 128):
            ob = outp.tile([128, D_MODEL], F32, tag="ob", name=f"ob{qc}")
            for e in range(2):
                po = pp.tile([128, 512], F32, tag="ab"[e], name=f"po{qc}_{e}")
                for i in range(2):
                    nc.tensor.matmul(
                        po,
                        ust[i][:, qc * 128:(qc + 1) * 128],
                        wo[i][:, e * 512:(e + 1) * 512],
                        start=(i == 0), stop=(i == 1))
                nc.vector.tensor_copy(out=ob[:, e * 512:(e + 1) * 512], in_=po)
            nc.sync.dma_start(out=out_d[qc * 128:(qc + 1) * 128, :], in_=ob)


# ------------------------------------------------------------- build + run
def _build():
    if "nc" in _cache:
        return _cache["nc"]
    nc = bacc.Bacc("TRN2", target_bir_lowering=False, debug=False)
    att_np_dt = mybir.dt.np(ATT_DT)
    ins = {
        "xt": nc.dram_tensor("xt", [D_MODEL, S], F32R, kind="ExternalInput").ap(),
        "wq": nc.dram_tensor("wq", [D_MODEL, DH], F32R, kind="ExternalInput").ap(),
        "wk": nc.dram_tensor("wk", [D_MODEL, DH], F32R, kind="ExternalInput").ap(),
        "wv": nc.dram_tensor("wv", [D_MODEL, DH], F32R, kind="ExternalInput").ap(),
        "wo": nc.dram_tensor("wo", [DH, D_MODEL], F32R, kind="ExternalInput").ap(),
        "expb": nc.dram_tensor("expb", [HPC, KC, TBL], ATT_DT,
                               kind="ExternalInput").ap(),
    }
    outs = {
        "out": nc.dram_tensor("out", [S, D_MODEL], F32, kind="ExternalOutput").ap(),
    }
    with tile.TileContext(nc) as tc:
        mha_body(tc, outs, ins)
    nc.compile()
    _cache["nc"] = nc
    return nc


TRACE = False
LAST = {}


def kernel(inputs, Wq, Wk, Wv, Wo, rel_emb):
    inputs = np.asarray(inputs, dtype=np.float32)
    Wq = np.asarray(Wq, dtype=np.float32)
    Wk = np.asarray(Wk, dtype=np.float32)
    Wv = np.asarray(Wv, dtype=np.float32)
    Wo = np.asarray(Wo, dtype=np.float32)
    rel_emb = np.asarray(rel_emb, dtype=np.float32)

    nc = _build()
    att_np_dt = mybir.dt.np(ATT_DT)

    ebt = _expbias_tables(rel_emb)  # [16, 128, TBL] f32
    in_maps = []
    for c in range(N_CORES):
        b, g = c // (N_CORES // B), c % (N_CORES // B)
        hs = slice(g * DH, (g + 1) * DH)
        in_maps.append({
            "xt": np.ascontiguousarray(inputs[b].T),
            "wq": np.ascontiguousarray(Wq[:, hs]),
            "wk": np.ascontiguousarray(Wk[:, hs]),
            "wv": np.ascontiguousarray(Wv[:, hs]),
            "wo": np.ascontiguousarray(Wo[hs, :]),
            "expb": np.ascontiguousarray(
                ebt[g * HPC:(g + 1) * HPC]).astype(att_np_dt),
        })

    res = run_bass_kernel_spmd(
        nc, in_maps, core_ids=list(range(N_CORES)), trace=TRACE)
    LAST["res"] = res

    out = np.zeros((B, S, D_MODEL), dtype=np.float64)
    for c in range(N_CORES):
        b = c // (N_CORES // B)
        out[b] += res.results[c]["out"].astype(np.float64)
    return out.astype(np.float32)



# revision 23
# speedup vs baseline: 80.5546x; 80.5546x over previous
"""T5-style MultiHeadAttention (relative position bias) on 8 Trainium2 cores.

Sharding: core c = (b, g) with b = c // 4 (batch), g = c % 4 (head group of 4
heads).  Each core computes q/k/v projections for its 4 heads, attention with
the relative-position bias, and a partial output projection (rows of Wo for
its heads).  Host sums the 4 partials per batch element.

Key layout choices (per core):
  - x is passed transposed: xT [1024, 2048] so projections contract over
    partitions directly.
  - Fused projection pass: each xT tile [128, 512] streamed from HBM exactly
    once feeds Q, K (as moving operand) AND V (its [128,128] column blocks as
    stationary operands), so there is no second x pass and half the phase-1
    DMA traffic disappears.
  - Q_t, K_t stored as [d, seq] (d on partitions); scores computed
    *transposed* as S_t[k, q] = K_t^T-stationary matmul, so that exp(S_t) can
    be used directly as the stationary operand of the attn@V matmul (which
    contracts over k = partitions).
  - Softmax denominator Z[q] = sum_k exp(S_t[k, q]) falls out of the attn@V
    matmul for free via a ones-column appended to V (M=65 per head).
  - No max-subtraction: scores are O(50) at most, exp stays finite in fp32
    and bf16.
  - Relative-position bias applied multiplicatively after exp:
    exp(s + bias) = exp(s) * exp(bias).  bias[k, q] = v_h[k - q + 2047] is
    Toeplitz, so exp(bias) tiles are overlapping windows of a single
    [128, 3968] sliding table per head, precomputed on host:
      T_h[p, i] = exp(v_h[p + 3967 - i])
    and the tile for (k0 = kc*128, q0 = qb*512) is T_h[:, base:base+512] with
    base = 1920 - k0 + q0.
  - Both heads of a pair share one [128, 1024] PSUM score tile spanning two
    banks, so one Exp activation covers both heads (1038 ns vs 2x 612 ns:
    the ACT engine is the attention-phase bottleneck).
  - All bias multiplies run on the DVE (bf16 2x mode, 327 ns); the GpSimd
    tensor_mul offload (1111 ns each) stalls its iterations and is gone.
  - Output projection runs inline per q block (qb outer, head-pair inner)
    and DMAs straight from PSUM, overlapping the next block's attention.
"""

import numpy as np
from contextlib import ExitStack

import concourse.bass as bass
import concourse.tile as tile
from concourse import bacc, mybir
from concourse.bass_utils import run_bass_kernel_spmd

# ---------------------------------------------------------------- constants
B, S, D_MODEL, N_HEADS, D_KV = 2, 2048, 1024, 16, 64
NUM_BUCKETS, MAX_DIST = 32, 128
N_CORES = 8
HPC = N_HEADS // (N_CORES // B)  # heads per core = 4
DH = HPC * D_KV                  # 256 d-cols per core
TBL = 3968                       # exp-bias sliding table width
QB = 512                         # q block (free dim of score tiles)
KC = 128                         # k chunk (partition dim of score tiles)

F32 = mybir.dt.float32
F32R = mybir.dt.float32r
BF16 = mybir.dt.bfloat16
AF = mybir.ActivationFunctionType

# attention-probability dtype: BF16 (fast DVE 2x) or F32 (accurate, 1x DVE)
ATT_DT = BF16

_cache = {}


# ------------------------------------------------------------- host helpers
def _rel_bucket(d):
    """Bucket of relative position d = k - q (bidirectional T5), numpy fp32
    mirror of the jax reference."""
    nb = NUM_BUCKETS // 2
    n = -d
    ret = (n < 0).astype(np.int32) * nb
    n = np.abs(n)
    max_exact = nb // 2
    is_small = n < max_exact
    nf = np.maximum(n, 1).astype(np.float32)
    val = (
        np.log(nf / np.float32(max_exact))
        / np.float32(np.log(MAX_DIST / max_exact))
        * np.float32(nb - max_exact)
    ).astype(np.int32) + max_exact
    val = np.minimum(val, nb - 1)
    return ret + np.where(is_small, n, val)


def _expbias_tables(rel_emb):
    """[N_HEADS, 128, TBL] exp-bias sliding tables (float32)."""
    d = np.arange(-(S - 1), S)  # k - q in [-2047, 2047]
    buck = _rel_bucket(d)  # [4095]
    vals = rel_emb[buck, :].astype(np.float32)  # [4095, H]
    idx = np.arange(KC)[:, None] + (TBL - 1) - np.arange(TBL)[None, :]
    t = np.exp(vals[idx, :])  # [128, TBL, H]
    return np.ascontiguousarray(np.transpose(t, (2, 0, 1)))


# ------------------------------------------------------------- kernel body
def mha_body(tc, outs, ins, ckpt=None):
    nc = tc.nc
    ctx = ExitStack()
    xt_d = ins["xt"].bitcast(F32R)        # [1024, 2048]
    wq_d = ins["wq"].bitcast(F32R)        # [1024, 256]
    wk_d = ins["wk"].bitcast(F32R)        # [1024, 256]
    wv_d = ins["wv"].bitcast(F32R)        # [1024, 256]
    wo_d = ins["wo"].bitcast(F32R)        # [256, 1024]
    eb_d = ins["expb"]      # [HPC, 128, TBL] ATT_DT
    out_d = outs["out"]     # [2048, 1024] f32

    att_np = ATT_DT
    DKN = D_MODEL // 128    # 8 contraction chunks
    NQ = S // QB            # 4 q blocks
    NK = S // KC            # 16 k chunks

    with ctx:
        const = ctx.enter_context(tc.tile_pool(name="const", bufs=1))

        # ---- persistent SBUF tensors
        qt = [const.tile([128, S], F32R, tag=f"qt{i}", name=f"qt{i}") for i in range(2)]
        kt = [const.tile([128, S], F32R, tag=f"kt{i}", name=f"kt{i}") for i in range(2)]
        # V with a ones column per head: [k, 4*65]; bf16 (AV stationary)
        vsb = [const.tile([128, HPC * 65], att_np, tag=f"v{i}", name=f"v{i}") for i in range(NK)]
        # normalized attention outputs, head-pairs stacked on partitions
        ust = [const.tile([128, S], F32R, tag=f"ust{i}", name=f"ust{i}") for i in range(2)]
        wo = [const.tile([128, D_MODEL], F32R, tag=f"wo{i}", name=f"wo{i}") for i in range(2)]
        # rows 192-255 of Wo again, at partition base 0: the final q block's
        # projection contracts the staging tile (partitions 0-63) against it
        wo1lo = const.tile([64, D_MODEL], F32R, tag="wo1lo", name="wo1lo")
        ebs = [const.tile([128, TBL], att_np, tag=f"eb{j}", name=f"eb{j}")
               for j in range(HPC)]

        # ---- flat pools (no scoped release: pool-release barriers idle the
        # PE >3.4us at phase boundaries and drop the HAM clock to 1.2GHz)
        wpool = ctx.enter_context(tc.tile_pool(name="wqkv", bufs=1))
        xtp = ctx.enter_context(tc.tile_pool(name="xts", bufs=8))
        esp = ctx.enter_context(tc.tile_pool(name="es", bufs=4))
        esbp = ctx.enter_context(tc.tile_pool(name="esb", bufs=4))
        rzp = ctx.enter_context(tc.tile_pool(name="rz", bufs=2))
        # one PSUM pool, 4 tags x 2 bufs = 8 banks, shared by all phases
        # (no psum pool release barriers).  Tags a,b hold [128,1024]
        # two-bank tiles in phase 2 (scores) and [128,512] tiles elsewhere.
        pp = ctx.enter_context(tc.tile_pool(name="pp", bufs=2, space="PSUM"))

        # weights live concatenated along the free dim: w*[:, dk*256:+256]
        # is contraction chunk dk.  Loaded in two half DMAs each (dk 0-3,
        # dk 4-7) — wide rearranged DMAs cost one HWDGE slot instead of 12.
        wqs = wpool.tile([128, DKN * DH], F32R, tag="wqs", name="wqs")
        wks = wpool.tile([128, DKN * DH], F32R, tag="wks", name="wks")
        wvs = wpool.tile([128, DKN * DH], F32R, tag="wvs", name="wvs")
        wq = [wqs[:, i * DH:(i + 1) * DH] for i in range(DKN)]
        wk = [wks[:, i * DH:(i + 1) * DH] for i in range(DKN)]
        wv = [wvs[:, i * DH:(i + 1) * DH] for i in range(DKN)]

        def _whalf(dst, src_d, h):
            half = DKN // 2
            nc.sync.dma_start(
                out=dst[:, h * half * DH:(h + 1) * half * DH]
                .rearrange("p (c d) -> p c d", c=half),
                in_=src_d[h * half * 128:(h + 1) * half * 128, :]
                .rearrange("(c p) d -> p c d", p=128))

        # DMA issue order is queue order: first-needed first.  wq half, then
        # the first xt tile (issued in the loop below), then the wk/wv
        # halves; second halves mid-qb0, bias tables late in phase 1, Wo
        # behind them.
        _whalf(wqs, wq_d, 0)

        # ================= phase 1: fused q/k/v projections =================
        # PSUM tag budget (16 KiB/partition = 8 banks): tag "s" holds the
        # [128,1024] two-bank score tiles in phase 2, so its slot is 4 KiB;
        # with bufs=2 that is 8 KiB.  Tags "c"/"d" hold one-bank tiles with
        # bufs=2: 4 KiB each.  Phase 1 packs q/k/v partials into the same
        # three tags.
        for qb in range(NQ):
            pq = [pp.tile([128, QB], F32, tag="s", name=f"pq{m}_{qb}")
                  for m in range(2)]
            pk = [pp.tile([128, QB], F32, tag=t, name=f"pk{m}_{qb}")
                  for m, t in ((0, "c"), (1, "d"))]
            # Q/K pass: stream this q block's 8 xt tiles; they stay resident
            # in the 8-deep pool so the V pass below reuses them with no
            # second DMA.  (PSUM accumulation groups are bank-granular, so
            # the four V chains cannot run concurrently with pq/pk — they
            # get their own bank slots right after.)
            xts = []
            for dk in range(DKN):
                xtt = xtp.tile([128, QB], F32R, tag="xts", name=f"xt_{qb}_{dk}")
                xts.append(xtt)
                nc.sync.dma_start(
                    out=xtt,
                    in_=xt_d[dk * 128:(dk + 1) * 128, qb * QB:(qb + 1) * QB])
                if qb == 0 and dk == 0:
                    _whalf(wks, wk_d, 0)
                    _whalf(wvs, wv_d, 0)
                if qb == 0 and dk == 1:
                    _whalf(wqs, wq_d, 1)
                    _whalf(wks, wk_d, 1)
                    _whalf(wvs, wv_d, 1)
                for m in range(2):
                    nc.tensor.matmul(
                        pq[m], wq[dk][:, m * 128:(m + 1) * 128], xtt,
                        start=(dk == 0), stop=(dk == DKN - 1))
                    nc.tensor.matmul(
                        pk[m], wk[dk][:, m * 128:(m + 1) * 128], xtt,
                        start=(dk == 0), stop=(dk == DKN - 1))
            # V pass from the resident xt tiles, two chains in flight
            pvs = []
            for s in range(4):
                pv = pp.tile([128, DH], F32, tag="cd"[s % 2],
                             name=f"pv{s}_{qb}")
                pvs.append(pv)
                for dk in range(DKN):
                    nc.tensor.matmul(
                        pv, xts[dk][:, s * 128:(s + 1) * 128], wv[dk],
                        start=(dk == 0), stop=(dk == DKN - 1))
            # bias tables are first read when qb0's scores hit the exp, right
            # after phase 1 — issued mid/late so their 2.9 us transfers don't
            # starve the phase-1 xt stream
            if qb == 1:
                nc.sync.dma_start(out=ebs[0], in_=eb_d[0])
                nc.sync.dma_start(out=ebs[1], in_=eb_d[1])
            if qb == 2:
                nc.sync.dma_start(out=ebs[2], in_=eb_d[2])
                nc.sync.dma_start(out=ebs[3], in_=eb_d[3])
            if qb == 3:
                for i in range(2):
                    nc.sync.dma_start(out=wo[i], in_=wo_d[i * 128:(i + 1) * 128, :])
                nc.sync.dma_start(out=wo1lo, in_=wo_d[192:256, :])
            for m in range(2):
                nc.scalar.copy(out=qt[m][:, qb * QB:(qb + 1) * QB], in_=pq[m])
                nc.scalar.copy(out=kt[m][:, qb * QB:(qb + 1) * QB], in_=pk[m])
            for s in range(4):
                kc = qb * 4 + s
                v3 = vsb[kc].rearrange("p (h c) -> p h c", h=HPC)
                nc.scalar.copy(
                    out=v3[:, :, 0:64],
                    in_=pvs[s].rearrange("p (h c) -> p h c", h=HPC))
                nc.vector.memset(v3[:, :, 64:65], 1.0)

        # ============ phase 2+3+4: attention + output projection ============
        # qb outer so both head pairs of a q block finish together and the
        # output projection for that block overlaps the next block's
        # attention.  Score matmuls of a pair use disjoint PE row groups
        # (rows 0-63 / 64-127); both land in one [128, 1024] two-bank PSUM
        # tile so a single Exp activation serves the pair.
        outp = ctx.enter_context(tc.tile_pool(name="outsb", bufs=3))
        last_stg = [None]  # [64,512] staging tile of the final (qb,hp) block

        def emit_po(qb):
            # output projection for q block qb; PSUM drained to SBUF
            # alternating between GpSimd and DVE so the drain keeps pace
            # with the PE, then DMA'd out.  The final block reads the second
            # head pair's lower half straight from the staging tile (third
            # K=64 matmul) instead of waiting for its ust DMA.
            for qc in range(qb * 4, qb * 4 + 4):
                ob = outp.tile([128, D_MODEL], F32, tag="ob", name=f"ob{qc}")
                for e in range(2):
                    po = pp.tile([128, 512], F32, tag="s", name=f"po{qc}_{e}")
                    es_ = slice(e * 512, (e + 1) * 512)
                    qs = slice(qc * 128, (qc + 1) * 128)
                    nc.tensor.matmul(po, ust[0][:, qs], wo[0][:, es_],
                                     start=True, stop=False)
                    if last_stg[0] is None:
                        nc.tensor.matmul(po, ust[1][:, qs], wo[1][:, es_],
                                         start=False, stop=True)
                    else:
                        nc.tensor.matmul(po, ust[1][0:64, qs],
                                         wo[1][0:64, es_],
                                         start=False, stop=False)
                        ls = slice((qc - qb * 4) * 128, (qc - qb * 4 + 1) * 128)
                        nc.tensor.matmul(po, last_stg[0][:, ls],
                                         wo1lo[:, es_],
                                         start=False, stop=True)
                    # GpSimd cannot read PSUM; DVE has slack for the drain
                    nc.vector.tensor_copy(out=ob[:, e * 512:(e + 1) * 512],
                                          in_=po)
                nc.sync.dma_start(out=out_d[qc * 128:(qc + 1) * 128, :],
                                  in_=ob)

        pending_po = []
        for qb in range(NQ):
            for hp in range(2):
                pus = [pp.tile([65, QB], F32, tag=t, name=f"pu{j}_{hp}_{qb}")
                       for j, t in ((0, "c"), (1, "d"))]
                for kc in range(NK):
                    base = (TBL - S) - kc * 128 + qb * QB
                    ps = pp.tile([128, 2 * QB], F32, tag="s",
                                 name=f"ps_{hp}_{qb}_{kc}")
                    for j in range(2):
                        prow = slice(j * 64, j * 64 + 64)
                        nc.tensor.matmul(
                            ps[:, j * QB:(j + 1) * QB],
                            kt[hp][prow, kc * 128:(kc + 1) * 128],
                            qt[hp][prow, qb * QB:(qb + 1) * QB],
                            start=True, stop=True)
                    es = esp.tile([128, 2 * QB], att_np, tag="es",
                                  name=f"es_{hp}_{qb}_{kc}")
                    nc.scalar.activation(out=es, in_=ps, func=AF.Exp)
                    esbs = []
                    for j in range(2):
                        esb = esbp.tile([128, QB], att_np, tag=f"esb{j}",
                                        name=f"esb{j}_{hp}_{qb}_{kc}")
                        nc.vector.tensor_mul(
                            esb, es[:, j * QB:(j + 1) * QB],
                            ebs[hp * 2 + j][:, base:base + QB])
                        esbs.append(esb)
                    for j in range(2):
                        h = hp * 2 + j
                        nc.tensor.matmul(
                            pus[j], vsb[kc][:, h * 65:(h + 1) * 65], esbs[j],
                            start=(kc == 0), stop=(kc == NK - 1))
                # normalize U[d, q] / Z[q]; Z = row 64 of pu
                for j in range(2):
                    rz = rzp.tile([1, QB], F32, tag=f"rz{j}",
                                  name=f"rz{j}_{hp}_{qb}")
                    nc.vector.reciprocal(out=rz, in_=pus[j][64:65, :])
                    rzb = rzp.tile([64, QB], F32, tag=f"rzb{j}",
                                   name=f"rzb{j}_{hp}_{qb}")
                    nc.gpsimd.partition_broadcast(rzb, rz, channels=64)
                    if j == 0:
                        nc.vector.tensor_mul(
                            ust[hp][0:64, qb * QB:(qb + 1) * QB],
                            pus[j][0:64, :], rzb)
                    else:
                        # DVE lanes are partition-locked; write via a [64,512]
                        # staging tile then DMA to rows 64-127
                        stg = rzp.tile([64, QB], F32R, tag="stg",
                                       name=f"stg{hp}_{qb}")
                        nc.vector.tensor_mul(stg, pus[j][0:64, :], rzb)
                        if qb == NQ - 1 and hp == 1:
                            last_stg[0] = stg
                        else:
                            nc.sync.dma_start(
                                out=ust[hp][64:128, qb * QB:(qb + 1) * QB],
                                in_=stg)

                # the projection for block qb-1 is emitted here, between the
                # two head pairs of block qb, so its ust staging DMA has long
                # completed and the PE (which executes in issue order) never
                # stalls on it
                if hp == 0 and pending_po:
                    emit_po(pending_po.pop(0))
            pending_po.append(qb)
        while pending_po:
            emit_po(pending_po.pop(0))


# ------------------------------------------------------------- build + run
def _build():
    if "nc" in _cache:
        return _cache["nc"]
    nc = bacc.Bacc("TRN2", target_bir_lowering=False, debug=False)
    ins = {
        "xt": nc.dram_tensor("xt", [D_MODEL, S], F32R, kind="ExternalInput").ap(),
        "wq": nc.dram_tensor("wq", [D_MODEL, DH], F32R, kind="ExternalInput").ap(),
        "wk": nc.dram_tensor("wk", [D_MODEL, DH], F32R, kind="ExternalInput").ap(),
        "wv": nc.dram_tensor("wv", [D_MODEL, DH], F32R, kind="ExternalInput").ap(),
        "wo": nc.dram_tensor("wo", [DH, D_MODEL], F32R, kind="ExternalInput").ap(),
        "expb": nc.dram_tensor("expb", [HPC, KC, TBL], ATT_DT,
                               kind="ExternalInput").ap(),
    }
    outs = {
        "out": nc.dram_tensor("out", [S, D_MODEL], F32, kind="ExternalOutput").ap(),
    }
    with tile.TileContext(nc) as tc:
        mha_body(tc, outs, ins)
    nc.compile()
    _cache["nc"] = nc
    return nc


TRACE = False
LAST = {}


def kernel(inputs, Wq, Wk, Wv, Wo, rel_emb):
    inputs = np.asarray(inputs, dtype=np.float32)
    Wq = np.asarray(Wq, dtype=np.float32)
    Wk = np.asarray(Wk, dtype=np.float32)
    Wv = np.asarray(Wv, dtype=np.float32)
    Wo = np.asarray(Wo, dtype=np.float32)
    rel_emb = np.asarray(rel_emb, dtype=np.float32)

    nc = _build()
    att_np_dt = mybir.dt.np(ATT_DT)

    ebt = _expbias_tables(rel_emb)  # [16, 128, TBL] f32
    in_maps = []
    for c in range(N_CORES):
        b, g = c // (N_CORES // B), c % (N_CORES // B)
        hs = slice(g * DH, (g + 1) * DH)
        in_maps.append({
            "xt": np.ascontiguousarray(inputs[b].T),
            "wq": np.ascontiguousarray(Wq[:, hs]),
            "wk": np.ascontiguousarray(Wk[:, hs]),
            "wv": np.ascontiguousarray(Wv[:, hs]),
            "wo": np.ascontiguousarray(Wo[hs, :]),
            "expb": np.ascontiguousarray(
                ebt[g * HPC:(g + 1) * HPC]).astype(att_np_dt),
        })

    res = run_bass_kernel_spmd(
        nc, in_maps, core_ids=list(range(N_CORES)), trace=TRACE)
    LAST["res"] = res

    out = np.zeros((B, S, D_MODEL), dtype=np.float64)
    for c in range(N_CORES):
        b = c // (N_CORES // B)
        out[b] += res.results[c]["out"].astype(np.float64)
    return out.astype(np.float32)


# revision 27
# speedup vs baseline: 139.5313x; 1.7321x over previous
"""T5-style MultiHeadAttention (relative position bias) on 8 Trainium2 cores.

Sharding: core c = (b, g) with b = c // 4 (batch), g = c % 4 (head group of 4
heads).  Each core computes q/k/v projections for its 4 heads, attention with
the relative-position bias, and a partial output projection (rows of Wo for
its heads).  Host sums the 4 partials per batch element.

Key layout choices (per core):
  - x is passed transposed: xT [1024, 2048] so projections contract over
    partitions directly.
  - Fused projection pass: each xT tile [128, 512] streamed from HBM exactly
    once feeds Q, K (as moving operand) AND V (its [128,128] column blocks as
    stationary operands), so there is no second x pass and half the phase-1
    DMA traffic disappears.
  - Q_t, K_t stored as [d, seq] (d on partitions); scores computed
    *transposed* as S_t[k, q] = K_t^T-stationary matmul, so that exp(S_t) can
    be used directly as the stationary operand of the attn@V matmul (which
    contracts over k = partitions).
  - Softmax denominator Z[q] = sum_k exp(S_t[k, q]) falls out of the attn@V
    matmul for free via a ones-column appended to V (M=65 per head).
  - No max-subtraction: scores are O(50) at most, exp stays finite in fp32
    and bf16.
  - Relative-position bias applied multiplicatively after exp:
    exp(s + bias) = exp(s) * exp(bias).  bias[k, q] = v_h[k - q + 2047] is
    Toeplitz, so exp(bias) tiles are overlapping windows of a single
    [128, 3968] sliding table per head, precomputed on host:
      T_h[p, i] = exp(v_h[p + 3967 - i])
    and the tile for (k0 = kc*128, q0 = qb*512) is T_h[:, base:base+512] with
    base = 1920 - k0 + q0.
  - Both heads of a pair share one [128, 1024] PSUM score tile spanning two
    banks, so one Exp activation covers both heads (1038 ns vs 2x 612 ns:
    the ACT engine is the attention-phase bottleneck).
  - All bias multiplies run on the DVE (bf16 2x mode, 327 ns); the GpSimd
    tensor_mul offload (1111 ns each) stalls its iterations and is gone.
  - Output projection runs inline per q block (qb outer, head-pair inner)
    and DMAs straight from PSUM, overlapping the next block's attention.
"""

import numpy as np
from contextlib import ExitStack

import concourse.bass as bass
import concourse.tile as tile
from concourse import bacc, mybir
from concourse.bass_utils import run_bass_kernel_spmd

# ---------------------------------------------------------------- constants
B, S, D_MODEL, N_HEADS, D_KV = 2, 2048, 1024, 16, 64
NUM_BUCKETS, MAX_DIST = 32, 128
N_CORES = 8
HPC = N_HEADS // (N_CORES // B)  # heads per core = 4
DH = HPC * D_KV                  # 256 d-cols per core
TBL = 3968                       # exp-bias sliding table width
QB = 512                         # q block (free dim of score tiles)
KC = 128                         # k chunk (partition dim of score tiles)

F32 = mybir.dt.float32
F32R = mybir.dt.float32r
BF16 = mybir.dt.bfloat16
AF = mybir.ActivationFunctionType

# attention-probability dtype: BF16 (fast DVE 2x) or F32 (accurate, 1x DVE)
ATT_DT = BF16

_cache = {}


# ------------------------------------------------------------- host helpers
def _rel_bucket(d):
    """Bucket of relative position d = k - q (bidirectional T5), numpy fp32
    mirror of the jax reference."""
    nb = NUM_BUCKETS // 2
    n = -d
    ret = (n < 0).astype(np.int32) * nb
    n = np.abs(n)
    max_exact = nb // 2
    is_small = n < max_exact
    nf = np.maximum(n, 1).astype(np.float32)
    val = (
        np.log(nf / np.float32(max_exact))
        / np.float32(np.log(MAX_DIST / max_exact))
        * np.float32(nb - max_exact)
    ).astype(np.int32) + max_exact
    val = np.minimum(val, nb - 1)
    return ret + np.where(is_small, n, val)


def _expbias_tables(rel_emb):
    """[N_HEADS, 128, TBL] exp-bias sliding tables (float32)."""
    d = np.arange(-(S - 1), S)  # k - q in [-2047, 2047]
    buck = _rel_bucket(d)  # [4095]
    vals = rel_emb[buck, :].astype(np.float32)  # [4095, H]
    idx = np.arange(KC)[:, None] + (TBL - 1) - np.arange(TBL)[None, :]
    t = np.exp(vals[idx, :])  # [128, TBL, H]
    return np.ascontiguousarray(np.transpose(t, (2, 0, 1)))


# ------------------------------------------------------------- kernel body
NXT = D_MODEL * S            # 2M f32: xT
NW = D_MODEL * DH            # 256K f32 each: wq, wk, wv, wo
EBW = HPC * KC * TBL // 2    # exp-bias tables, bf16 pairs packed as f32
NBLOB = NXT + 4 * NW + EBW


def mha_body(tc, outs, ins, ckpt=None):
    nc = tc.nc
    ctx = ExitStack()
    # ALL inputs ride in one flat f32 blob: every extra NEFF argument costs
    # ~100 us of per-execute dispatch in the runtime.  The bf16 bias tables
    # are bit-packed in the f32 tail and bitcast back here.
    xw = ins["xw"]
    xt_d = xw[0:NXT].rearrange("(a b) -> a b", b=S).bitcast(F32R)
    wq_d = xw[NXT:NXT + NW].rearrange("(a b) -> a b", b=DH).bitcast(F32R)
    wk_d = xw[NXT + NW:NXT + 2 * NW].rearrange("(a b) -> a b", b=DH).bitcast(F32R)
    wv_d = xw[NXT + 2 * NW:NXT + 3 * NW].rearrange("(a b) -> a b", b=DH).bitcast(F32R)
    wo_d = xw[NXT + 3 * NW:NXT + 4 * NW].rearrange("(a b) -> a b", b=D_MODEL).bitcast(F32R)
    eb_d = (xw[NXT + 4 * NW:NBLOB].bitcast(ATT_DT)
            .rearrange("(h p t) -> h p t", p=KC, t=TBL))  # [HPC, 128, TBL]
    out_d = outs["out"]     # [2048, 1024] f32

    att_np = ATT_DT
    DKN = D_MODEL // 128    # 8 contraction chunks
    NQ = S // QB            # 4 q blocks
    NK = S // KC            # 16 k chunks

    with ctx:
        const = ctx.enter_context(tc.tile_pool(name="const", bufs=1))

        # ---- persistent SBUF tensors
        qt = [const.tile([128, S], F32R, tag=f"qt{i}", name=f"qt{i}") for i in range(2)]
        kt = [const.tile([128, S], F32R, tag=f"kt{i}", name=f"kt{i}") for i in range(2)]
        # V with a ones column per head: [k, 4*65]; bf16 (AV stationary)
        vsb = [const.tile([128, HPC * 65], att_np, tag=f"v{i}", name=f"v{i}") for i in range(NK)]
        # normalized attention outputs, head-pairs stacked on partitions
        ust = [const.tile([128, S], F32R, tag=f"ust{i}", name=f"ust{i}") for i in range(2)]
        wo = [const.tile([128, D_MODEL], F32R, tag=f"wo{i}", name=f"wo{i}") for i in range(2)]
        # rows 192-255 of Wo again, at partition base 0: the final q block's
        # projection contracts the staging tile (partitions 0-63) against it
        wo1lo = const.tile([64, D_MODEL], F32R, tag="wo1lo", name="wo1lo")
        ebs = [const.tile([128, TBL], att_np, tag=f"eb{j}", name=f"eb{j}")
               for j in range(HPC)]

        # ---- flat pools (no scoped release: pool-release barriers idle the
        # PE >3.4us at phase boundaries and drop the HAM clock to 1.2GHz)
        wpool = ctx.enter_context(tc.tile_pool(name="wqkv", bufs=1))
        xtp = ctx.enter_context(tc.tile_pool(name="xts", bufs=8))
        esp = ctx.enter_context(tc.tile_pool(name="es", bufs=4))
        esbp = ctx.enter_context(tc.tile_pool(name="esb", bufs=4))
        rzp = ctx.enter_context(tc.tile_pool(name="rz", bufs=2))
        # one PSUM pool, 4 tags x 2 bufs = 8 banks, shared by all phases
        # (no psum pool release barriers).  Tags a,b hold [128,1024]
        # two-bank tiles in phase 2 (scores) and [128,512] tiles elsewhere.
        pp = ctx.enter_context(tc.tile_pool(name="pp", bufs=2, space="PSUM"))

        # weights live concatenated along the free dim: w*[:, dk*256:+256]
        # is contraction chunk dk.  Loaded in two half DMAs each (dk 0-3,
        # dk 4-7) — wide rearranged DMAs cost one HWDGE slot instead of 12.
        wqs = wpool.tile([128, DKN * DH], F32R, tag="wqs", name="wqs")
        wks = wpool.tile([128, DKN * DH], F32R, tag="wks", name="wks")
        wvs = wpool.tile([128, DKN * DH], F32R, tag="wvs", name="wvs")
        wq = [wqs[:, i * DH:(i + 1) * DH] for i in range(DKN)]
        wk = [wks[:, i * DH:(i + 1) * DH] for i in range(DKN)]
        wv = [wvs[:, i * DH:(i + 1) * DH] for i in range(DKN)]

        def _whalf(dst, src_d, h):
            half = DKN // 2
            nc.sync.dma_start(
                out=dst[:, h * half * DH:(h + 1) * half * DH]
                .rearrange("p (c d) -> p c d", c=half),
                in_=src_d[h * half * 128:(h + 1) * half * 128, :]
                .rearrange("(c p) d -> p c d", p=128))

        # DMA issue order is queue order: first-needed first.  wq half, then
        # the first xt tile (issued in the loop below), then the wk/wv
        # halves; second halves mid-qb0, bias tables late in phase 1, Wo
        # behind them.
        _whalf(wqs, wq_d, 0)

        # ================= phase 1: fused q/k/v projections =================
        # PSUM tag budget (16 KiB/partition = 8 banks): tag "s" holds the
        # [128,1024] two-bank score tiles in phase 2, so its slot is 4 KiB;
        # with bufs=2 that is 8 KiB.  Tags "c"/"d" hold one-bank tiles with
        # bufs=2: 4 KiB each.  Phase 1 packs q/k/v partials into the same
        # three tags.
        for qb in range(NQ):
            pq = [pp.tile([128, QB], F32, tag="s", name=f"pq{m}_{qb}")
                  for m in range(2)]
            pk = [pp.tile([128, QB], F32, tag=t, name=f"pk{m}_{qb}")
                  for m, t in ((0, "c"), (1, "d"))]
            # Q/K pass: stream this q block's 8 xt tiles; they stay resident
            # in the 8-deep pool so the V pass below reuses them with no
            # second DMA.  (PSUM accumulation groups are bank-granular, so
            # the four V chains cannot run concurrently with pq/pk — they
            # get their own bank slots right after.)
            xts = []
            for dk in range(DKN):
                xtt = xtp.tile([128, QB], F32R, tag="xts", name=f"xt_{qb}_{dk}")
                xts.append(xtt)
                nc.sync.dma_start(
                    out=xtt,
                    in_=xt_d[dk * 128:(dk + 1) * 128, qb * QB:(qb + 1) * QB])
                if qb == 0 and dk == 0:
                    _whalf(wks, wk_d, 0)
                    _whalf(wvs, wv_d, 0)
                if qb == 0 and dk == 1:
                    _whalf(wqs, wq_d, 1)
                    _whalf(wks, wk_d, 1)
                    _whalf(wvs, wv_d, 1)
                for m in range(2):
                    nc.tensor.matmul(
                        pq[m], wq[dk][:, m * 128:(m + 1) * 128], xtt,
                        start=(dk == 0), stop=(dk == DKN - 1))
                    nc.tensor.matmul(
                        pk[m], wk[dk][:, m * 128:(m + 1) * 128], xtt,
                        start=(dk == 0), stop=(dk == DKN - 1))
            # V pass from the resident xt tiles, two chains in flight
            pvs = []
            for s in range(4):
                pv = pp.tile([128, DH], F32, tag="cd"[s % 2],
                             name=f"pv{s}_{qb}")
                pvs.append(pv)
                for dk in range(DKN):
                    nc.tensor.matmul(
                        pv, xts[dk][:, s * 128:(s + 1) * 128], wv[dk],
                        start=(dk == 0), stop=(dk == DKN - 1))
            # bias tables are first read when qb0's scores hit the exp, right
            # after phase 1 — issued mid/late so their 2.9 us transfers don't
            # starve the phase-1 xt stream
            if qb == 1:
                nc.sync.dma_start(out=ebs[0], in_=eb_d[0])
                nc.sync.dma_start(out=ebs[1], in_=eb_d[1])
            if qb == 2:
                nc.sync.dma_start(out=ebs[2], in_=eb_d[2])
                nc.sync.dma_start(out=ebs[3], in_=eb_d[3])
            if qb == 3:
                for i in range(2):
                    nc.sync.dma_start(out=wo[i], in_=wo_d[i * 128:(i + 1) * 128, :])
                nc.sync.dma_start(out=wo1lo, in_=wo_d[192:256, :])
            for m in range(2):
                nc.scalar.copy(out=qt[m][:, qb * QB:(qb + 1) * QB], in_=pq[m])
                nc.scalar.copy(out=kt[m][:, qb * QB:(qb + 1) * QB], in_=pk[m])
            for s in range(4):
                kc = qb * 4 + s
                v3 = vsb[kc].rearrange("p (h c) -> p h c", h=HPC)
                nc.scalar.copy(
                    out=v3[:, :, 0:64],
                    in_=pvs[s].rearrange("p (h c) -> p h c", h=HPC))
                nc.vector.memset(v3[:, :, 64:65], 1.0)

        # ============ phase 2+3+4: attention + output projection ============
        # qb outer so both head pairs of a q block finish together and the
        # output projection for that block overlaps the next block's
        # attention.  Score matmuls of a pair use disjoint PE row groups
        # (rows 0-63 / 64-127); both land in one [128, 1024] two-bank PSUM
        # tile so a single Exp activation serves the pair.
        outp = ctx.enter_context(tc.tile_pool(name="outsb", bufs=3))
        last_stg = [None]  # [64,512] staging tile of the final (qb,hp) block

        def emit_po(qb):
            # output projection for q block qb; PSUM drained to SBUF
            # alternating between GpSimd and DVE so the drain keeps pace
            # with the PE, then DMA'd out.  The final block reads the second
            # head pair's lower half straight from the staging tile (third
            # K=64 matmul) instead of waiting for its ust DMA.
            for qc in range(qb * 4, qb * 4 + 4):
                ob = outp.tile([128, D_MODEL], F32, tag="ob", name=f"ob{qc}")
                for e in range(2):
                    po = pp.tile([128, 512], F32, tag="s", name=f"po{qc}_{e}")
                    es_ = slice(e * 512, (e + 1) * 512)
                    qs = slice(qc * 128, (qc + 1) * 128)
                    nc.tensor.matmul(po, ust[0][:, qs], wo[0][:, es_],
                                     start=True, stop=False)
                    if last_stg[0] is None:
                        nc.tensor.matmul(po, ust[1][:, qs], wo[1][:, es_],
                                         start=False, stop=True)
                    else:
                        nc.tensor.matmul(po, ust[1][0:64, qs],
                                         wo[1][0:64, es_],
                                         start=False, stop=False)
                        ls = slice((qc - qb * 4) * 128, (qc - qb * 4 + 1) * 128)
                        nc.tensor.matmul(po, last_stg[0][:, ls],
                                         wo1lo[:, es_],
                                         start=False, stop=True)
                    # GpSimd cannot read PSUM; DVE has slack for the drain
                    nc.vector.tensor_copy(out=ob[:, e * 512:(e + 1) * 512],
                                          in_=po)
                nc.sync.dma_start(out=out_d[qc * 128:(qc + 1) * 128, :],
                                  in_=ob)

        pending_po = []
        for qb in range(NQ):
            for hp in range(2):
                pus = [pp.tile([65, QB], F32, tag=t, name=f"pu{j}_{hp}_{qb}")
                       for j, t in ((0, "c"), (1, "d"))]
                for kc in range(NK):
                    base = (TBL - S) - kc * 128 + qb * QB
                    ps = pp.tile([128, 2 * QB], F32, tag="s",
                                 name=f"ps_{hp}_{qb}_{kc}")
                    for j in range(2):
                        prow = slice(j * 64, j * 64 + 64)
                        nc.tensor.matmul(
                            ps[:, j * QB:(j + 1) * QB],
                            kt[hp][prow, kc * 128:(kc + 1) * 128],
                            qt[hp][prow, qb * QB:(qb + 1) * QB],
                            start=True, stop=True)
                    es = esp.tile([128, 2 * QB], att_np, tag="es",
                                  name=f"es_{hp}_{qb}_{kc}")
                    nc.scalar.activation(out=es, in_=ps, func=AF.Exp)
                    esbs = []
                    for j in range(2):
                        esb = esbp.tile([128, QB], att_np, tag=f"esb{j}",
                                        name=f"esb{j}_{hp}_{qb}_{kc}")
                        nc.vector.tensor_mul(
                            esb, es[:, j * QB:(j + 1) * QB],
                            ebs[hp * 2 + j][:, base:base + QB])
                        esbs.append(esb)
                    for j in range(2):
                        h = hp * 2 + j
                        nc.tensor.matmul(
                            pus[j], vsb[kc][:, h * 65:(h + 1) * 65], esbs[j],
                            start=(kc == 0), stop=(kc == NK - 1))
                # normalize U[d, q] / Z[q]; Z = row 64 of pu
                for j in range(2):
                    rz = rzp.tile([1, QB], F32, tag=f"rz{j}",
                                  name=f"rz{j}_{hp}_{qb}")
                    nc.vector.reciprocal(out=rz, in_=pus[j][64:65, :])
                    rzb = rzp.tile([64, QB], F32, tag=f"rzb{j}",
                                   name=f"rzb{j}_{hp}_{qb}")
                    nc.gpsimd.partition_broadcast(rzb, rz, channels=64)
                    if j == 0:
                        nc.vector.tensor_mul(
                            ust[hp][0:64, qb * QB:(qb + 1) * QB],
                            pus[j][0:64, :], rzb)
                    else:
                        # DVE lanes are partition-locked; write via a [64,512]
                        # staging tile then DMA to rows 64-127
                        stg = rzp.tile([64, QB], F32R, tag="stg",
                                       name=f"stg{hp}_{qb}")
                        nc.vector.tensor_mul(stg, pus[j][0:64, :], rzb)
                        if qb == NQ - 1 and hp == 1:
                            last_stg[0] = stg
                        else:
                            nc.sync.dma_start(
                                out=ust[hp][64:128, qb * QB:(qb + 1) * QB],
                                in_=stg)

                # the projection for block qb-1 is emitted here, between the
                # two head pairs of block qb, so its ust staging DMA has long
                # completed and the PE (which executes in issue order) never
                # stalls on it
                if hp == 0 and pending_po:
                    emit_po(pending_po.pop(0))
            pending_po.append(qb)
        while pending_po:
            emit_po(pending_po.pop(0))


# ------------------------------------------------------------- build + run
def _build():
    if "nc" in _cache:
        return _cache["nc"]
    nc = bacc.Bacc("TRN2", target_bir_lowering=False, debug=False)
    ins = {
        "xw": nc.dram_tensor("xw", [NXT + 4 * NW], F32,
                             kind="ExternalInput").ap(),
        "expb": nc.dram_tensor("expb", [HPC, KC, TBL], ATT_DT,
                               kind="ExternalInput").ap(),
    }
    outs = {
        "out": nc.dram_tensor("out", [S, D_MODEL], F32, kind="ExternalOutput").ap(),
    }
    with tile.TileContext(nc) as tc:
        mha_body(tc, outs, ins)
    nc.compile()
    _cache["nc"] = nc
    return nc


TRACE = False
LAST = {}


def kernel(inputs, Wq, Wk, Wv, Wo, rel_emb):
    inputs = np.asarray(inputs, dtype=np.float32)
    Wq = np.asarray(Wq, dtype=np.float32)
    Wk = np.asarray(Wk, dtype=np.float32)
    Wv = np.asarray(Wv, dtype=np.float32)
    Wo = np.asarray(Wo, dtype=np.float32)
    rel_emb = np.asarray(rel_emb, dtype=np.float32)

    nc = _build()
    att_np_dt = mybir.dt.np(ATT_DT)

    ebt = _expbias_tables(rel_emb)  # [16, 128, TBL] f32
    in_maps = []
    for c in range(N_CORES):
        b, g = c // (N_CORES // B), c % (N_CORES // B)
        hs = slice(g * DH, (g + 1) * DH)
        xw = np.concatenate([
            inputs[b].T.ravel(),
            Wq[:, hs].ravel(),
            Wk[:, hs].ravel(),
            Wv[:, hs].ravel(),
            Wo[hs, :].ravel(),
        ]).astype(np.float32)
        in_maps.append({
            "xw": xw,
            "expb": np.ascontiguousarray(
                ebt[g * HPC:(g + 1) * HPC]).astype(att_np_dt),
        })

    res = run_bass_kernel_spmd(
        nc, in_maps, core_ids=list(range(N_CORES)), trace=TRACE)
    LAST["res"] = res

    out = np.zeros((B, S, D_MODEL), dtype=np.float64)
    for c in range(N_CORES):
        b = c // (N_CORES // B)
        out[b] += res.results[c]["out"].astype(np.float64)
    return out.astype(np.float32)


# revision 30
# speedup vs baseline: 159.3596x; 1.1421x over previous
"""T5-style MultiHeadAttention (relative position bias) on 8 Trainium2 cores.

Sharding: core c = (b, g) with b = c // 4 (batch), g = c % 4 (head group of 4
heads).  Each core computes q/k/v projections for its 4 heads, attention with
the relative-position bias, and a partial output projection (rows of Wo for
its heads).  Host sums the 4 partials per batch element.

Key layout choices (per core):
  - x is passed transposed: xT [1024, 2048] so projections contract over
    partitions directly.
  - Fused projection pass: each xT tile [128, 512] streamed from HBM exactly
    once feeds Q, K (as moving operand) AND V (its [128,128] column blocks as
    stationary operands), so there is no second x pass and half the phase-1
    DMA traffic disappears.
  - Q_t, K_t stored as [d, seq] (d on partitions); scores computed
    *transposed* as S_t[k, q] = K_t^T-stationary matmul, so that exp(S_t) can
    be used directly as the stationary operand of the attn@V matmul (which
    contracts over k = partitions).
  - Softmax denominator Z[q] = sum_k exp(S_t[k, q]) falls out of the attn@V
    matmul for free via a ones-column appended to V (M=65 per head).
  - No max-subtraction: scores are O(50) at most, exp stays finite in fp32
    and bf16.
  - Relative-position bias applied multiplicatively after exp:
    exp(s + bias) = exp(s) * exp(bias).  bias[k, q] = v_h[k - q + 2047] is
    Toeplitz, so exp(bias) tiles are overlapping windows of a single
    [128, 3968] sliding table per head, precomputed on host:
      T_h[p, i] = exp(v_h[p + 3967 - i])
    and the tile for (k0 = kc*128, q0 = qb*512) is T_h[:, base:base+512] with
    base = 1920 - k0 + q0.
  - Both heads of a pair share one [128, 1024] PSUM score tile spanning two
    banks, so one Exp activation covers both heads (1038 ns vs 2x 612 ns:
    the ACT engine is the attention-phase bottleneck).
  - All bias multiplies run on the DVE (bf16 2x mode, 327 ns); the GpSimd
    tensor_mul offload (1111 ns each) stalls its iterations and is gone.
  - Output projection runs inline per q block (qb outer, head-pair inner)
    and DMAs straight from PSUM, overlapping the next block's attention.
"""

import numpy as np
from contextlib import ExitStack

import concourse.bass as bass
import concourse.tile as tile
from concourse import bacc, mybir
from concourse.bass_utils import run_bass_kernel_spmd

# ---------------------------------------------------------------- constants
B, S, D_MODEL, N_HEADS, D_KV = 2, 2048, 1024, 16, 64
NUM_BUCKETS, MAX_DIST = 32, 128
N_CORES = 8
HPC = N_HEADS // (N_CORES // B)  # heads per core = 4
DH = HPC * D_KV                  # 256 d-cols per core
TBL = 3968                       # exp-bias sliding table width
QB = 512                         # q block (free dim of score tiles)
KC = 128                         # k chunk (partition dim of score tiles)

F32 = mybir.dt.float32
F32R = mybir.dt.float32r
BF16 = mybir.dt.bfloat16
AF = mybir.ActivationFunctionType

# attention-probability dtype: BF16 (fast DVE 2x) or F32 (accurate, 1x DVE)
ATT_DT = BF16

_cache = {}


# ------------------------------------------------------------- host helpers
def _rel_bucket(d):
    """Bucket of relative position d = k - q (bidirectional T5), numpy fp32
    mirror of the jax reference."""
    nb = NUM_BUCKETS // 2
    n = -d
    ret = (n < 0).astype(np.int32) * nb
    n = np.abs(n)
    max_exact = nb // 2
    is_small = n < max_exact
    nf = np.maximum(n, 1).astype(np.float32)
    val = (
        np.log(nf / np.float32(max_exact))
        / np.float32(np.log(MAX_DIST / max_exact))
        * np.float32(nb - max_exact)
    ).astype(np.int32) + max_exact
    val = np.minimum(val, nb - 1)
    return ret + np.where(is_small, n, val)


def _expbias_tables(rel_emb):
    """[N_HEADS, 128, TBL] exp-bias sliding tables (float32)."""
    d = np.arange(-(S - 1), S)  # k - q in [-2047, 2047]
    buck = _rel_bucket(d)  # [4095]
    vals = rel_emb[buck, :].astype(np.float32)  # [4095, H]
    idx = np.arange(KC)[:, None] + (TBL - 1) - np.arange(TBL)[None, :]
    t = np.exp(vals[idx, :])  # [128, TBL, H]
    return np.ascontiguousarray(np.transpose(t, (2, 0, 1)))


# ------------------------------------------------------------- kernel body
NXT = D_MODEL * S            # 2M f32: xT
NW = D_MODEL * DH            # 256K f32 each: wq, wk, wv, wo
EBW = HPC * KC * TBL // 2    # exp-bias tables, bf16 pairs packed as f32
NBLOB = NXT + 4 * NW + EBW


def mha_body(tc, outs, ins, ckpt=None):
    nc = tc.nc
    ctx = ExitStack()
    # ALL inputs ride in one flat f32 blob: every extra NEFF argument costs
    # ~100 us of per-execute dispatch in the runtime.  The bf16 bias tables
    # are bit-packed in the f32 tail and bitcast back here.
    xw = ins["xw"]
    xt_d = xw[0:NXT].rearrange("(a b) -> a b", b=S).bitcast(F32R)
    wq_d = xw[NXT:NXT + NW].rearrange("(a b) -> a b", b=DH).bitcast(F32R)
    wk_d = xw[NXT + NW:NXT + 2 * NW].rearrange("(a b) -> a b", b=DH).bitcast(F32R)
    wv_d = xw[NXT + 2 * NW:NXT + 3 * NW].rearrange("(a b) -> a b", b=DH).bitcast(F32R)
    wo_d = xw[NXT + 3 * NW:NXT + 4 * NW].rearrange("(a b) -> a b", b=D_MODEL).bitcast(F32R)
    eb_d = (xw[NXT + 4 * NW:NBLOB].bitcast(ATT_DT)
            .rearrange("(h p t) -> h p t", p=KC, t=TBL))  # [HPC, 128, TBL]
    out_d = outs["out"]     # [2048, 1024] f32

    att_np = ATT_DT
    DKN = D_MODEL // 128    # 8 contraction chunks
    NQ = S // QB            # 4 q blocks
    NK = S // KC            # 16 k chunks

    with ctx:
        const = ctx.enter_context(tc.tile_pool(name="const", bufs=1))

        # ---- persistent SBUF tensors
        qt = [const.tile([128, S], F32R, tag=f"qt{i}", name=f"qt{i}") for i in range(2)]
        kt = [const.tile([128, S], F32R, tag=f"kt{i}", name=f"kt{i}") for i in range(2)]
        # V with a ones column per head: [k, 4*65]; bf16 (AV stationary)
        vsb = [const.tile([128, HPC * 65], att_np, tag=f"v{i}", name=f"v{i}") for i in range(NK)]
        # normalized attention outputs, head-pairs stacked on partitions
        ust = [const.tile([128, S], F32R, tag=f"ust{i}", name=f"ust{i}") for i in range(2)]
        wo = [const.tile([128, D_MODEL], F32R, tag=f"wo{i}", name=f"wo{i}") for i in range(2)]
        # rows 192-255 of Wo again, at partition base 0: the final q block's
        # projection contracts the staging tile (partitions 0-63) against it
        wo1lo = const.tile([64, D_MODEL], F32R, tag="wo1lo", name="wo1lo")
        ebs = [const.tile([128, TBL], att_np, tag=f"eb{j}", name=f"eb{j}")
               for j in range(HPC)]

        # ---- flat pools (no scoped release: pool-release barriers idle the
        # PE >3.4us at phase boundaries and drop the HAM clock to 1.2GHz)
        wpool = ctx.enter_context(tc.tile_pool(name="wqkv", bufs=1))
        xtp = ctx.enter_context(tc.tile_pool(name="xts", bufs=8))
        esp = ctx.enter_context(tc.tile_pool(name="es", bufs=4))
        esbp = ctx.enter_context(tc.tile_pool(name="esb", bufs=4))
        rzp = ctx.enter_context(tc.tile_pool(name="rz", bufs=2))
        # one PSUM pool, 4 tags x 2 bufs = 8 banks, shared by all phases
        # (no psum pool release barriers).  Tags a,b hold [128,1024]
        # two-bank tiles in phase 2 (scores) and [128,512] tiles elsewhere.
        pp = ctx.enter_context(tc.tile_pool(name="pp", bufs=2, space="PSUM"))

        # weights live concatenated along the free dim: w*[:, dk*256:+256]
        # is contraction chunk dk.  Loaded in two half DMAs each (dk 0-3,
        # dk 4-7) — wide rearranged DMAs cost one HWDGE slot instead of 12.
        wqs = wpool.tile([128, DKN * DH], F32R, tag="wqs", name="wqs")
        wks = wpool.tile([128, DKN * DH], F32R, tag="wks", name="wks")
        wvs = wpool.tile([128, DKN * DH], F32R, tag="wvs", name="wvs")
        wq = [wqs[:, i * DH:(i + 1) * DH] for i in range(DKN)]
        wk = [wks[:, i * DH:(i + 1) * DH] for i in range(DKN)]
        wv = [wvs[:, i * DH:(i + 1) * DH] for i in range(DKN)]

        def _whalf(dst, src_d, h):
            half = DKN // 2
            nc.sync.dma_start(
                out=dst[:, h * half * DH:(h + 1) * half * DH]
                .rearrange("p (c d) -> p c d", c=half),
                in_=src_d[h * half * 128:(h + 1) * half * 128, :]
                .rearrange("(c p) d -> p c d", p=128))

        # DMA issue order is queue order: first-needed first.  wq half, then
        # the first xt tile (issued in the loop below), then the wk/wv
        # halves; second halves mid-qb0, bias tables late in phase 1, Wo
        # behind them.
        _whalf(wqs, wq_d, 0)

        # ================= phase 1: fused q/k/v projections =================
        # PSUM tag budget (16 KiB/partition = 8 banks): tag "s" holds the
        # [128,1024] two-bank score tiles in phase 2, so its slot is 4 KiB;
        # with bufs=2 that is 8 KiB.  Tags "c"/"d" hold one-bank tiles with
        # bufs=2: 4 KiB each.  Phase 1 packs q/k/v partials into the same
        # three tags.
        for qb in range(NQ):
            # PSUM accumulation groups are bank-granular, so each of the 8
            # concurrent chains (pq0, pq1, pk0, pk1, pv0-3) gets its own
            # bank: the two-bank tag-s slots hold pq|pv pairs, tags c/d hold
            # pk in one buffer and a pv in the other.
            big = [pp.tile([128, 2 * QB], F32, tag="s", name=f"pqv{m}_{qb}")
                   for m in range(2)]
            pq = [big[m][:, 0:QB] for m in range(2)]
            pk = [pp.tile([128, QB], F32, tag=t, name=f"pk{m}_{qb}")
                  for m, t in ((0, "c"), (1, "d"))]
            pvs = [
                big[0][:, QB:QB + DH],
                big[1][:, QB:QB + DH],
                pp.tile([128, DH], F32, tag="c", name=f"pv2_{qb}"),
                pp.tile([128, DH], F32, tag="d", name=f"pv3_{qb}"),
            ]
            for dk in range(DKN):
                xtt = xtp.tile([128, QB], F32R, tag="xts", name=f"xt_{qb}_{dk}")
                nc.sync.dma_start(
                    out=xtt,
                    in_=xt_d[dk * 128:(dk + 1) * 128, qb * QB:(qb + 1) * QB])
                if qb == 0 and dk == 0:
                    _whalf(wks, wk_d, 0)
                    _whalf(wvs, wv_d, 0)
                if qb == 0 and dk == 1:
                    _whalf(wqs, wq_d, 1)
                    _whalf(wks, wk_d, 1)
                    _whalf(wvs, wv_d, 1)
                for m in range(2):
                    nc.tensor.matmul(
                        pq[m], wq[dk][:, m * 128:(m + 1) * 128], xtt,
                        start=(dk == 0), stop=(dk == DKN - 1))
                    nc.tensor.matmul(
                        pk[m], wk[dk][:, m * 128:(m + 1) * 128], xtt,
                        start=(dk == 0), stop=(dk == DKN - 1))
                for s in range(4):
                    nc.tensor.matmul(
                        pvs[s], xtt[:, s * 128:(s + 1) * 128], wv[dk],
                        start=(dk == 0), stop=(dk == DKN - 1))
            # bias tables are first read when qb0's scores hit the exp, right
            # after phase 1 — issued mid/late so their 2.9 us transfers don't
            # starve the phase-1 xt stream
            if qb == 1:
                nc.sync.dma_start(out=ebs[0], in_=eb_d[0])
                nc.sync.dma_start(out=ebs[1], in_=eb_d[1])
            if qb == 2:
                nc.sync.dma_start(out=ebs[2], in_=eb_d[2])
                nc.sync.dma_start(out=ebs[3], in_=eb_d[3])
            if qb == 3:
                for i in range(2):
                    nc.sync.dma_start(out=wo[i], in_=wo_d[i * 128:(i + 1) * 128, :])
                nc.sync.dma_start(out=wo1lo, in_=wo_d[192:256, :])
            for m in range(2):
                nc.scalar.copy(out=qt[m][:, qb * QB:(qb + 1) * QB], in_=pq[m])
                nc.scalar.copy(out=kt[m][:, qb * QB:(qb + 1) * QB], in_=pk[m])
            for s in range(4):
                kc = qb * 4 + s
                v3 = vsb[kc].rearrange("p (h c) -> p h c", h=HPC)
                nc.scalar.copy(
                    out=v3[:, :, 0:64],
                    in_=pvs[s].rearrange("p (h c) -> p h c", h=HPC))
                nc.vector.memset(v3[:, :, 64:65], 1.0)

        # ============ phase 2+3+4: attention + output projection ============
        # qb outer so both head pairs of a q block finish together and the
        # output projection for that block overlaps the next block's
        # attention.  Score matmuls of a pair use disjoint PE row groups
        # (rows 0-63 / 64-127); both land in one [128, 1024] two-bank PSUM
        # tile so a single Exp activation serves the pair.
        outp = ctx.enter_context(tc.tile_pool(name="outsb", bufs=3))
        last_stg = [None]  # [64,512] staging tile of the final (qb,hp) block

        def emit_po(qb):
            # output projection for q block qb; PSUM drained to SBUF
            # alternating between GpSimd and DVE so the drain keeps pace
            # with the PE, then DMA'd out.  The final block reads the second
            # head pair's lower half straight from the staging tile (third
            # K=64 matmul) instead of waiting for its ust DMA.
            for qc in range(qb * 4, qb * 4 + 4):
                ob = outp.tile([128, D_MODEL], F32, tag="ob", name=f"ob{qc}")
                for e in range(2):
                    po = pp.tile([128, 512], F32, tag="s", name=f"po{qc}_{e}")
                    es_ = slice(e * 512, (e + 1) * 512)
                    qs = slice(qc * 128, (qc + 1) * 128)
                    nc.tensor.matmul(po, ust[0][:, qs], wo[0][:, es_],
                                     start=True, stop=False)
                    if last_stg[0] is None:
                        nc.tensor.matmul(po, ust[1][:, qs], wo[1][:, es_],
                                         start=False, stop=True)
                    else:
                        nc.tensor.matmul(po, ust[1][0:64, qs],
                                         wo[1][0:64, es_],
                                         start=False, stop=False)
                        ls = slice((qc - qb * 4) * 128, (qc - qb * 4 + 1) * 128)
                        nc.tensor.matmul(po, last_stg[0][:, ls],
                                         wo1lo[:, es_],
                                         start=False, stop=True)
                    # GpSimd cannot read PSUM; DVE has slack for the drain
                    nc.vector.tensor_copy(out=ob[:, e * 512:(e + 1) * 512],
                                          in_=po)
                nc.sync.dma_start(out=out_d[qc * 128:(qc + 1) * 128, :],
                                  in_=ob)

        pending_po = []
        for qb in range(NQ):
            for hp in range(2):
                pus = [pp.tile([65, QB], F32, tag=t, name=f"pu{j}_{hp}_{qb}")
                       for j, t in ((0, "c"), (1, "d"))]
                for kc in range(NK):
                    base = (TBL - S) - kc * 128 + qb * QB
                    ps = pp.tile([128, 2 * QB], F32, tag="s",
                                 name=f"ps_{hp}_{qb}_{kc}")
                    for j in range(2):
                        prow = slice(j * 64, j * 64 + 64)
                        nc.tensor.matmul(
                            ps[:, j * QB:(j + 1) * QB],
                            kt[hp][prow, kc * 128:(kc + 1) * 128],
                            qt[hp][prow, qb * QB:(qb + 1) * QB],
                            start=True, stop=True)
                    es = esp.tile([128, 2 * QB], att_np, tag="es",
                                  name=f"es_{hp}_{qb}_{kc}")
                    nc.scalar.activation(out=es, in_=ps, func=AF.Exp)
                    esbs = []
                    for j in range(2):
                        esb = esbp.tile([128, QB], att_np, tag=f"esb{j}",
                                        name=f"esb{j}_{hp}_{qb}_{kc}")
                        nc.vector.tensor_mul(
                            esb, es[:, j * QB:(j + 1) * QB],
                            ebs[hp * 2 + j][:, base:base + QB])
                        esbs.append(esb)
                    for j in range(2):
                        h = hp * 2 + j
                        nc.tensor.matmul(
                            pus[j], vsb[kc][:, h * 65:(h + 1) * 65], esbs[j],
                            start=(kc == 0), stop=(kc == NK - 1))
                # normalize U[d, q] / Z[q]; Z = row 64 of pu
                for j in range(2):
                    rz = rzp.tile([1, QB], F32, tag=f"rz{j}",
                                  name=f"rz{j}_{hp}_{qb}")
                    nc.vector.reciprocal(out=rz, in_=pus[j][64:65, :])
                    rzb = rzp.tile([64, QB], F32, tag=f"rzb{j}",
                                   name=f"rzb{j}_{hp}_{qb}")
                    nc.gpsimd.partition_broadcast(rzb, rz, channels=64)
                    if j == 0:
                        nc.vector.tensor_mul(
                            ust[hp][0:64, qb * QB:(qb + 1) * QB],
                            pus[j][0:64, :], rzb)
                    else:
                        # DVE lanes are partition-locked; write via a [64,512]
                        # staging tile then DMA to rows 64-127
                        stg = rzp.tile([64, QB], F32R, tag="stg",
                                       name=f"stg{hp}_{qb}")
                        nc.vector.tensor_mul(stg, pus[j][0:64, :], rzb)
                        if qb == NQ - 1 and hp == 1:
                            last_stg[0] = stg
                        else:
                            nc.sync.dma_start(
                                out=ust[hp][64:128, qb * QB:(qb + 1) * QB],
                                in_=stg)

                # the projection for block qb-1 is emitted here, between the
                # two head pairs of block qb, so its ust staging DMA has long
                # completed and the PE (which executes in issue order) never
                # stalls on it
                if hp == 0 and pending_po:
                    emit_po(pending_po.pop(0))
            pending_po.append(qb)
        while pending_po:
            emit_po(pending_po.pop(0))


# ------------------------------------------------------------- build + run
def _build():
    if "nc" in _cache:
        return _cache["nc"]
    nc = bacc.Bacc("TRN2", target_bir_lowering=False, debug=False)
    ins = {
        "xw": nc.dram_tensor("xw", [NBLOB], F32, kind="ExternalInput").ap(),
    }
    outs = {
        "out": nc.dram_tensor("out", [S, D_MODEL], F32, kind="ExternalOutput").ap(),
    }
    with tile.TileContext(nc) as tc:
        mha_body(tc, outs, ins)
    nc.compile()
    _cache["nc"] = nc
    return nc


TRACE = False
LAST = {}


def kernel(inputs, Wq, Wk, Wv, Wo, rel_emb):
    inputs = np.asarray(inputs, dtype=np.float32)
    Wq = np.asarray(Wq, dtype=np.float32)
    Wk = np.asarray(Wk, dtype=np.float32)
    Wv = np.asarray(Wv, dtype=np.float32)
    Wo = np.asarray(Wo, dtype=np.float32)
    rel_emb = np.asarray(rel_emb, dtype=np.float32)

    nc = _build()
    att_np_dt = mybir.dt.np(ATT_DT)

    ebt = _expbias_tables(rel_emb)  # [16, 128, TBL] f32
    in_maps = []
    for c in range(N_CORES):
        b, g = c // (N_CORES // B), c % (N_CORES // B)
        hs = slice(g * DH, (g + 1) * DH)
        eb_bits = (np.ascontiguousarray(ebt[g * HPC:(g + 1) * HPC])
                   .astype(att_np_dt).ravel().view(np.float32))
        xw = np.concatenate([
            inputs[b].T.ravel(),
            Wq[:, hs].ravel(),
            Wk[:, hs].ravel(),
            Wv[:, hs].ravel(),
            Wo[hs, :].ravel(),
            eb_bits,
        ]).astype(np.float32)
        in_maps.append({"xw": xw})

    res = run_bass_kernel_spmd(
        nc, in_maps, core_ids=list(range(N_CORES)), trace=TRACE)
    LAST["res"] = res

    out = np.zeros((B, S, D_MODEL), dtype=np.float64)
    for c in range(N_CORES):
        b = c // (N_CORES // B)
        out[b] += res.results[c]["out"].astype(np.float64)
    return out.astype(np.float32)


# revision 35
# speedup vs baseline: 673.4080x; 4.2257x over previous
"""T5-style MultiHeadAttention (relative position bias) on 8 Trainium2 cores.

Sharding: core c = (b, g) with b = c // 4 (batch), g = c % 4 (head group of 4
heads).  Each core computes q/k/v projections for its 4 heads, attention with
the relative-position bias, and a partial output projection (rows of Wo for
its heads).  Host sums the 4 partials per batch element.

Key layout choices (per core):
  - x is passed transposed: xT [1024, 2048] so projections contract over
    partitions directly.
  - Fused projection pass: each xT tile [128, 512] streamed from HBM exactly
    once feeds Q, K (as moving operand) AND V (its [128,128] column blocks as
    stationary operands), so there is no second x pass and half the phase-1
    DMA traffic disappears.
  - Q_t, K_t stored as [d, seq] (d on partitions); scores computed
    *transposed* as S_t[k, q] = K_t^T-stationary matmul, so that exp(S_t) can
    be used directly as the stationary operand of the attn@V matmul (which
    contracts over k = partitions).
  - Softmax denominator Z[q] = sum_k exp(S_t[k, q]) falls out of the attn@V
    matmul for free via a ones-column appended to V (M=65 per head).
  - No max-subtraction: scores are O(50) at most, exp stays finite in fp32
    and bf16.
  - Relative-position bias applied multiplicatively after exp:
    exp(s + bias) = exp(s) * exp(bias).  bias[k, q] = v_h[k - q + 2047] is
    Toeplitz, so exp(bias) tiles are overlapping windows of a single
    [128, 3968] sliding table per head, precomputed on host:
      T_h[p, i] = exp(v_h[p + 3967 - i])
    and the tile for (k0 = kc*128, q0 = qb*512) is T_h[:, base:base+512] with
    base = 1920 - k0 + q0.
  - Both heads of a pair share one [128, 1024] PSUM score tile spanning two
    banks, so one Exp activation covers both heads (1038 ns vs 2x 612 ns:
    the ACT engine is the attention-phase bottleneck).
  - All bias multiplies run on the DVE (bf16 2x mode, 327 ns); the GpSimd
    tensor_mul offload (1111 ns each) stalls its iterations and is gone.
  - Output projection runs inline per q block (qb outer, head-pair inner)
    and DMAs straight from PSUM, overlapping the next block's attention.
"""

import numpy as np
from contextlib import ExitStack

import concourse.bass as bass
import concourse.tile as tile
from concourse import bacc, mybir
from concourse.bass_utils import run_bass_kernel_spmd

# ---------------------------------------------------------------- constants
B, S, D_MODEL, N_HEADS, D_KV = 2, 2048, 1024, 16, 64
NUM_BUCKETS, MAX_DIST = 32, 128
N_CORES = 8
HPC = N_HEADS // (N_CORES // B)  # heads per core = 4
DH = HPC * D_KV                  # 256 d-cols per core
TBL = 3968                       # exp-bias sliding table width
QB = 512                         # q block (free dim of score tiles)
KC = 128                         # k chunk (partition dim of score tiles)

F32 = mybir.dt.float32
F32R = mybir.dt.float32r
BF16 = mybir.dt.bfloat16
AF = mybir.ActivationFunctionType

# attention-probability dtype: BF16 (fast DVE 2x) or F32 (accurate, 1x DVE)
ATT_DT = BF16

_cache = {}


# ------------------------------------------------------------- host helpers
def _rel_bucket(d):
    """Bucket of relative position d = k - q (bidirectional T5), numpy fp32
    mirror of the jax reference."""
    nb = NUM_BUCKETS // 2
    n = -d
    ret = (n < 0).astype(np.int32) * nb
    n = np.abs(n)
    max_exact = nb // 2
    is_small = n < max_exact
    nf = np.maximum(n, 1).astype(np.float32)
    val = (
        np.log(nf / np.float32(max_exact))
        / np.float32(np.log(MAX_DIST / max_exact))
        * np.float32(nb - max_exact)
    ).astype(np.int32) + max_exact
    val = np.minimum(val, nb - 1)
    return ret + np.where(is_small, n, val)


def _expbias_tables(rel_emb):
    """[N_HEADS, 128, TBL] exp-bias sliding tables (float32)."""
    d = np.arange(-(S - 1), S)  # k - q in [-2047, 2047]
    buck = _rel_bucket(d)  # [4095]
    vals = rel_emb[buck, :].astype(np.float32)  # [4095, H]
    idx = np.arange(KC)[:, None] + (TBL - 1) - np.arange(TBL)[None, :]
    t = np.exp(vals[idx, :])  # [128, TBL, H]
    return np.ascontiguousarray(np.transpose(t, (2, 0, 1)))


# ------------------------------------------------------------- kernel body
NXT = D_MODEL * S            # 2M f32: xT
NW = D_MODEL * DH            # 256K f32 each: wq, wk, wv, wo
EBW = HPC * KC * TBL // 2    # exp-bias tables, bf16 pairs packed as f32
NBLOB = NXT + 4 * NW + EBW


def mha_body(tc, outs, ins, ckpt=None):
    nc = tc.nc
    ctx = ExitStack()
    # ALL inputs ride in one flat f32 blob: every extra NEFF argument costs
    # ~100 us of per-execute dispatch in the runtime.  The bf16 bias tables
    # are bit-packed in the f32 tail and bitcast back here.
    xw = ins["xw"]
    xt_d = xw[0:NXT].rearrange("(a b) -> a b", b=S).bitcast(F32R)
    wq_d = xw[NXT:NXT + NW].rearrange("(a b) -> a b", b=DH).bitcast(F32R)
    wk_d = xw[NXT + NW:NXT + 2 * NW].rearrange("(a b) -> a b", b=DH).bitcast(F32R)
    wv_d = xw[NXT + 2 * NW:NXT + 3 * NW].rearrange("(a b) -> a b", b=DH).bitcast(F32R)
    wo_d = xw[NXT + 3 * NW:NXT + 4 * NW].rearrange("(a b) -> a b", b=D_MODEL).bitcast(F32R)
    eb_d = (xw[NXT + 4 * NW:NBLOB].bitcast(ATT_DT)
            .rearrange("(h p t) -> h p t", p=KC, t=TBL))  # [HPC, 128, TBL]
    out_d = outs["out"]     # [2048, 1024] f32

    att_np = ATT_DT
    DKN = D_MODEL // 128    # 8 contraction chunks
    NQ = S // QB            # 4 q blocks
    NK = S // KC            # 16 k chunks

    with ctx:
        const = ctx.enter_context(tc.tile_pool(name="const", bufs=1))

        # ---- persistent SBUF tensors
        qt = [const.tile([128, S], F32R, tag=f"qt{i}", name=f"qt{i}") for i in range(2)]
        kt = [const.tile([128, S], F32R, tag=f"kt{i}", name=f"kt{i}") for i in range(2)]
        # V with a ones column per head: [k, 4*65]; bf16 (AV stationary)
        vsb = [const.tile([128, HPC * 65], att_np, tag=f"v{i}", name=f"v{i}") for i in range(NK)]
        # normalized attention outputs, head-pairs stacked on partitions
        ust = [const.tile([128, S], F32R, tag=f"ust{i}", name=f"ust{i}") for i in range(2)]
        wo = [const.tile([128, D_MODEL], F32R, tag=f"wo{i}", name=f"wo{i}") for i in range(2)]
        # rows 192-255 of Wo again, at partition base 0: the final q block's
        # projection contracts the staging tile (partitions 0-63) against it
        wo1lo = const.tile([64, D_MODEL], F32R, tag="wo1lo", name="wo1lo")
        ebs = [const.tile([128, TBL], att_np, tag=f"eb{j}", name=f"eb{j}")
               for j in range(HPC)]

        # ---- flat pools (no scoped release: pool-release barriers idle the
        # PE >3.4us at phase boundaries and drop the HAM clock to 1.2GHz)
        wpool = ctx.enter_context(tc.tile_pool(name="wqkv", bufs=1))
        xtp = ctx.enter_context(tc.tile_pool(name="xts", bufs=8))
        esp = ctx.enter_context(tc.tile_pool(name="es", bufs=4))
        esbp = ctx.enter_context(tc.tile_pool(name="esb", bufs=4))
        rzp = ctx.enter_context(tc.tile_pool(name="rz", bufs=2))
        # one PSUM pool, 4 tags x 2 bufs = 8 banks, shared by all phases
        # (no psum pool release barriers).  Tags a,b hold [128,1024]
        # two-bank tiles in phase 2 (scores) and [128,512] tiles elsewhere.
        pp = ctx.enter_context(tc.tile_pool(name="pp", bufs=2, space="PSUM"))

        # weights live concatenated along the free dim: w*[:, dk*256:+256]
        # is contraction chunk dk.  Loaded in two half DMAs each (dk 0-3,
        # dk 4-7) — wide rearranged DMAs cost one HWDGE slot instead of 12.
        wqs = wpool.tile([128, DKN * DH], F32R, tag="wqs", name="wqs")
        wks = wpool.tile([128, DKN * DH], F32R, tag="wks", name="wks")
        wvs = wpool.tile([128, DKN * DH], F32R, tag="wvs", name="wvs")
        wq = [wqs[:, i * DH:(i + 1) * DH] for i in range(DKN)]
        wk = [wks[:, i * DH:(i + 1) * DH] for i in range(DKN)]
        wv = [wvs[:, i * DH:(i + 1) * DH] for i in range(DKN)]

        def _whalf(dst, src_d, h):
            half = DKN // 2
            nc.sync.dma_start(
                out=dst[:, h * half * DH:(h + 1) * half * DH]
                .rearrange("p (c d) -> p c d", c=half),
                in_=src_d[h * half * 128:(h + 1) * half * 128, :]
                .rearrange("(c p) d -> p c d", p=128))

        # DMA issue order is queue order: first-needed first.  wq half, then
        # the first xt tile (issued in the loop below), then the wk/wv
        # halves; second halves mid-qb0, bias tables late in phase 1, Wo
        # behind them.
        _whalf(wqs, wq_d, 0)

        # ================= phase 1: fused q/k/v projections =================
        # PSUM tag budget (16 KiB/partition = 8 banks): tag "s" holds the
        # [128,1024] two-bank score tiles in phase 2, so its slot is 4 KiB;
        # with bufs=2 that is 8 KiB.  Tags "c"/"d" hold one-bank tiles with
        # bufs=2: 4 KiB each.  Phase 1 packs q/k/v partials into the same
        # three tags.
        for qb in range(NQ):
            # PSUM accumulation groups are bank-granular, so each of the 8
            # concurrent chains (pq0, pq1, pk0, pk1, pv0-3) gets its own
            # bank: the two-bank tag-s slots hold pq|pv pairs, tags c/d hold
            # pk in one buffer and a pv in the other.
            big = [pp.tile([128, 2 * QB], F32, tag="s", name=f"pqv{m}_{qb}")
                   for m in range(2)]
            pq = [big[m][:, 0:QB] for m in range(2)]
            pk = [pp.tile([128, QB], F32, tag=t, name=f"pk{m}_{qb}")
                  for m, t in ((0, "c"), (1, "d"))]
            pvs = [
                big[0][:, QB:QB + DH],
                big[1][:, QB:QB + DH],
                pp.tile([128, DH], F32, tag="c", name=f"pv2_{qb}"),
                pp.tile([128, DH], F32, tag="d", name=f"pv3_{qb}"),
            ]
            for dk in range(DKN):
                xtt = xtp.tile([128, QB], F32R, tag="xts", name=f"xt_{qb}_{dk}")
                nc.sync.dma_start(
                    out=xtt,
                    in_=xt_d[dk * 128:(dk + 1) * 128, qb * QB:(qb + 1) * QB])
                if qb == 0 and dk == 0:
                    _whalf(wks, wk_d, 0)
                    _whalf(wvs, wv_d, 0)
                if qb == 0 and dk == 1:
                    _whalf(wqs, wq_d, 1)
                    _whalf(wks, wk_d, 1)
                    _whalf(wvs, wv_d, 1)
                for m in range(2):
                    nc.tensor.matmul(
                        pq[m], wq[dk][:, m * 128:(m + 1) * 128], xtt,
                        start=(dk == 0), stop=(dk == DKN - 1))
                    nc.tensor.matmul(
                        pk[m], wk[dk][:, m * 128:(m + 1) * 128], xtt,
                        start=(dk == 0), stop=(dk == DKN - 1))
                for s in range(4):
                    nc.tensor.matmul(
                        pvs[s], xtt[:, s * 128:(s + 1) * 128], wv[dk],
                        start=(dk == 0), stop=(dk == DKN - 1))
            # bias tables are first read when qb0's scores hit the exp, right
            # after phase 1 — issued mid/late so their 2.9 us transfers don't
            # starve the phase-1 xt stream
            if qb == 1:
                nc.sync.dma_start(out=ebs[0], in_=eb_d[0])
                nc.sync.dma_start(out=ebs[1], in_=eb_d[1])
            if qb == 2:
                nc.sync.dma_start(out=ebs[2], in_=eb_d[2])
                nc.sync.dma_start(out=ebs[3], in_=eb_d[3])
            if qb == 3:
                for i in range(2):
                    nc.sync.dma_start(out=wo[i], in_=wo_d[i * 128:(i + 1) * 128, :])
                nc.sync.dma_start(out=wo1lo, in_=wo_d[192:256, :])
            for m in range(2):
                nc.scalar.copy(out=qt[m][:, qb * QB:(qb + 1) * QB], in_=pq[m])
                nc.scalar.copy(out=kt[m][:, qb * QB:(qb + 1) * QB], in_=pk[m])
            for s in range(4):
                kc = qb * 4 + s
                v3 = vsb[kc].rearrange("p (h c) -> p h c", h=HPC)
                nc.scalar.copy(
                    out=v3[:, :, 0:64],
                    in_=pvs[s].rearrange("p (h c) -> p h c", h=HPC))
                nc.vector.memset(v3[:, :, 64:65], 1.0)

        # ============ phase 2+3+4: attention + output projection ============
        # qb outer so both head pairs of a q block finish together and the
        # output projection for that block overlaps the next block's
        # attention.  Score matmuls of a pair use disjoint PE row groups
        # (rows 0-63 / 64-127); both land in one [128, 1024] two-bank PSUM
        # tile so a single Exp activation serves the pair.
        outp = ctx.enter_context(tc.tile_pool(name="outsb", bufs=3))
        last_stg = [None]  # [64,512] staging tile of the final (qb,hp) block

        def emit_po_unit(qc):
            # one 128-row slab of the output projection, in a two-bank tag-s
            # tile (the column halves are separate accumulation banks), one
            # DVE drain, one DMA.  Units are interleaved into a LATER
            # block's kc loop so the PE absorbs them in its slack while the
            # ACT (the attention bottleneck) never goes idle.  The final
            # block reads the second head pair's lower half straight from
            # the staging tile (third K=64 matmul) instead of waiting for
            # its ust DMA.
            ob = outp.tile([128, D_MODEL], F32, tag="ob", name=f"ob{qc}")
            po = pp.tile([128, 2 * QB], F32, tag="s", name=f"po{qc}")
            qs = slice(qc * 128, (qc + 1) * 128)
            for e in range(2):
                pe_ = po[:, e * QB:(e + 1) * QB]
                es_ = slice(e * 512, (e + 1) * 512)
                nc.tensor.matmul(pe_, ust[0][:, qs], wo[0][:, es_],
                                 start=True, stop=False)
                if last_stg[0] is None:
                    nc.tensor.matmul(pe_, ust[1][:, qs], wo[1][:, es_],
                                     start=False, stop=True)
                else:
                    nc.tensor.matmul(pe_, ust[1][0:64, qs],
                                     wo[1][0:64, es_],
                                     start=False, stop=False)
                    ls = slice((qc % 4) * 128, (qc % 4 + 1) * 128)
                    nc.tensor.matmul(pe_, last_stg[0][:, ls],
                                     wo1lo[:, es_],
                                     start=False, stop=True)
            # GpSimd cannot read PSUM; DVE has slack for the drain
            nc.vector.tensor_copy(out=ob, in_=po)
            nc.sync.dma_start(out=out_d[qc * 128:(qc + 1) * 128, :], in_=ob)

        def emit_po_unit_cd(qc):
            # same projection slab, but in the second buffers of tags c/d
            # (free mid-loop: the live pus tiles hold the first buffers).
            # These banks are outside the tag-s score rotation, so this
            # variant costs the ACT pipeline nothing.
            ob = outp.tile([128, D_MODEL], F32, tag="ob", name=f"ob{qc}")
            qs = slice(qc * 128, (qc + 1) * 128)
            for e, tg in ((0, "c"), (1, "d")):
                po = pp.tile([128, QB], F32, tag=tg, name=f"po{qc}_{e}")
                es_ = slice(e * 512, (e + 1) * 512)
                nc.tensor.matmul(po, ust[0][:, qs], wo[0][:, es_],
                                 start=True, stop=False)
                if last_stg[0] is None:
                    nc.tensor.matmul(po, ust[1][:, qs], wo[1][:, es_],
                                     start=False, stop=True)
                else:
                    nc.tensor.matmul(po, ust[1][0:64, qs], wo[1][0:64, es_],
                                     start=False, stop=False)
                    ls = slice((qc % 4) * 128, (qc % 4 + 1) * 128)
                    nc.tensor.matmul(po, last_stg[0][:, ls], wo1lo[:, es_],
                                     start=False, stop=True)
                nc.vector.tensor_copy(out=ob[:, es_], in_=po)
            nc.sync.dma_start(out=out_d[qc * 128:(qc + 1) * 128, :], in_=ob)

        pending_po = []
        for qb in range(NQ):
            for hp in range(2):
                pus = [pp.tile([65, QB], F32, tag=t, name=f"pu{j}_{hp}_{qb}")
                       for j, t in ((0, "c"), (1, "d"))]
                for kc in range(NK):
                    # two projection-unit slots per kc loop, away from the
                    # block boundaries: one in the tag-s rotation, one in
                    # the free c/d buffers
                    if kc == 5 and pending_po:
                        emit_po_unit(pending_po.pop(0))
                    if kc == 11 and pending_po:
                        emit_po_unit_cd(pending_po.pop(0))
                    base = (TBL - S) - kc * 128 + qb * QB
                    ps = pp.tile([128, 2 * QB], F32, tag="s",
                                 name=f"ps_{hp}_{qb}_{kc}")
                    for j in range(2):
                        prow = slice(j * 64, j * 64 + 64)
                        nc.tensor.matmul(
                            ps[:, j * QB:(j + 1) * QB],
                            kt[hp][prow, kc * 128:(kc + 1) * 128],
                            qt[hp][prow, qb * QB:(qb + 1) * QB],
                            start=True, stop=True)
                    es = esp.tile([128, 2 * QB], att_np, tag="es",
                                  name=f"es_{hp}_{qb}_{kc}")
                    nc.scalar.activation(out=es, in_=ps, func=AF.Exp)
                    esbs = []
                    for j in range(2):
                        esb = esbp.tile([128, QB], att_np, tag=f"esb{j}",
                                        name=f"esb{j}_{hp}_{qb}_{kc}")
                        nc.vector.tensor_mul(
                            esb, es[:, j * QB:(j + 1) * QB],
                            ebs[hp * 2 + j][:, base:base + QB])
                        esbs.append(esb)
                    for j in range(2):
                        h = hp * 2 + j
                        nc.tensor.matmul(
                            pus[j], vsb[kc][:, h * 65:(h + 1) * 65], esbs[j],
                            start=(kc == 0), stop=(kc == NK - 1))
                # normalize U[d, q] / Z[q]; Z = row 64 of pu
                for j in range(2):
                    rz = rzp.tile([1, QB], F32, tag=f"rz{j}",
                                  name=f"rz{j}_{hp}_{qb}")
                    nc.vector.reciprocal(out=rz, in_=pus[j][64:65, :])
                    rzb = rzp.tile([64, QB], F32, tag=f"rzb{j}",
                                   name=f"rzb{j}_{hp}_{qb}")
                    nc.gpsimd.partition_broadcast(rzb, rz, channels=64)
                    if j == 0:
                        nc.vector.tensor_mul(
                            ust[hp][0:64, qb * QB:(qb + 1) * QB],
                            pus[j][0:64, :], rzb)
                    else:
                        # DVE lanes are partition-locked; write via a [64,512]
                        # staging tile then DMA to rows 64-127
                        stg = rzp.tile([64, QB], F32R, tag="stg",
                                       name=f"stg{hp}_{qb}")
                        nc.vector.tensor_mul(stg, pus[j][0:64, :], rzb)
                        if qb == NQ - 1 and hp == 1:
                            last_stg[0] = stg
                        else:
                            nc.sync.dma_start(
                                out=ust[hp][64:128, qb * QB:(qb + 1) * QB],
                                in_=stg)

            # queue this block's four projection slabs; they are emitted two
            # per kc loop starting a full head-pair later, so the ust
            # staging DMA has long completed before the PE reaches them
            pending_po.extend(range(qb * 4, qb * 4 + 4))
        while pending_po:
            emit_po_unit(pending_po.pop(0))


# ------------------------------------------------------------- build + run
def _build():
    if "nc" in _cache:
        return _cache["nc"]
    nc = bacc.Bacc("TRN2", target_bir_lowering=False, debug=False)
    ins = {
        "xw": nc.dram_tensor("xw", [NBLOB], F32, kind="ExternalInput").ap(),
    }
    outs = {
        "out": nc.dram_tensor("out", [S, D_MODEL], F32, kind="ExternalOutput").ap(),
    }
    with tile.TileContext(nc) as tc:
        mha_body(tc, outs, ins)
    nc.compile()
    _cache["nc"] = nc
    return nc


TRACE = False
LAST = {}


def kernel(inputs, Wq, Wk, Wv, Wo, rel_emb):
    inputs = np.asarray(inputs, dtype=np.float32)
    Wq = np.asarray(Wq, dtype=np.float32)
    Wk = np.asarray(Wk, dtype=np.float32)
    Wv = np.asarray(Wv, dtype=np.float32)
    Wo = np.asarray(Wo, dtype=np.float32)
    rel_emb = np.asarray(rel_emb, dtype=np.float32)

    nc = _build()
    att_np_dt = mybir.dt.np(ATT_DT)

    ebt = _expbias_tables(rel_emb)  # [16, 128, TBL] f32
    in_maps = []
    for c in range(N_CORES):
        b, g = c // (N_CORES // B), c % (N_CORES // B)
        hs = slice(g * DH, (g + 1) * DH)
        eb_bits = (np.ascontiguousarray(ebt[g * HPC:(g + 1) * HPC])
                   .astype(att_np_dt).ravel().view(np.float32))
        xw = np.concatenate([
            inputs[b].T.ravel(),
            Wq[:, hs].ravel(),
            Wk[:, hs].ravel(),
            Wv[:, hs].ravel(),
            Wo[hs, :].ravel(),
            eb_bits,
        ]).astype(np.float32)
        in_maps.append({"xw": xw})

    res = run_bass_kernel_spmd(
        nc, in_maps, core_ids=list(range(N_CORES)), trace=TRACE)
    LAST["res"] = res

    out = np.zeros((B, S, D_MODEL), dtype=np.float64)
    for c in range(N_CORES):
        b = c // (N_CORES // B)
        out[b] += res.results[c]["out"].astype(np.float64)
    return out.astype(np.float32)
